# revision 28
# baseline (speedup 1.0000x reference)
"""GPT-2 style causal attention block (B=4, S=2048, E=1024, H=16, D=64) on
8 TRN2 NeuronCores.

Sharding: batch(4) x head-half(2) -> 8 cores, zero on-device communication.
Core c handles batch b=c//2 and heads h0=(c%2)*8 .. h0+7. Each core computes
its qkv column block, attention for its 8 heads, and a partial c_proj
(its 512 rows of w_proj). The partial outputs per batch are summed on the
host during unshard (which also applies the 1/1024 weight-scale and b_proj).

fp8 DoubleRow usage (error-free hi/lo residual splits unless noted):
- qkv: X^T and weights pre-split on the host into e4m3 (hi, lo) packed
  [128, 4, 2, cols]; each DR matmul contracts 256 embedding rows at 0.5
  cyc/col; hh+hl+lh gives 6 column-passes vs bf16's 8. Weights are 32x
  pre-scaled (e4m3 normal range), so Q^T/K^T/V are carried 32-scaled.
- scores: Q^T/K^T quantized to e4m3 (plain, ~1.1e-2 added rel err) and
  DMA-regrouped so head-dim contracts as a DR [32, 2] pack at 0.5 cyc/col.
- c_proj: A^T carried as a 32-scaled e4m3 (hi, lo) pair, w_proj 32x
  pre-scaled and split on the host; ct-pairs contract 256 rows per DR
  matmul (3 split terms = 6 passes vs bf16's 8). The 1/1024 descale and
  b_proj land in the host-side gather.

Attention (per head, q-chunks of 1024): scores^T[k, q] via W-stationary
matmuls, exp on ACT (the dominant ACT cost, ~135us: it bounds how much
the other engines may carry), causality by computing only k<=q 128-tiles
plus a gpsimd affine_select on each diagonal 128-block. attn@V is
REORIENTED: out[q, d] per (q-tile, kt) with P as the stationary operand
pays 64 columns instead of 128 -> half the PE cost of the [d, q] form.
The 8 q-tile accumulators of a chunk pack into ONE PSUM bank [128, 8, 64];
softmax denominators accumulate via rank-1 ones-column matmuls into a
second bank, giving one batched reciprocal per (head, chunk) and a
normalization that is folded into the PSUM-drain copy (per-partition
scalar). A^T is then rebuilt per head-pair by PE transposes (identity
matmul) with psum drains split across DVE/Pool, writing the e4m3 hi/lo
pair that c_proj consumes.

Scheduling: attention bodies priority-boosted over filler (qkv pairs 2-3,
V tiles 8-15, c_proj tiles) which is interleaved into the exp-bound
stretches; during the DMA-paced ramp the qkv groups borrow the idle
attention PSUM banks. PSUM accumulates f32; copies avoid ACT entirely
(exp saturates it) and alternate DVE/Pool.
"""

import re

import ml_dtypes
import numpy as np

import concourse.mybir as mybir
import concourse.tile as tile
from concourse import bacc
from concourse.bass_utils import run_bass_kernel_spmd
from concourse.vector_clock import ScopedClock

F32 = mybir.dt.float32
BF16 = mybir.dt.bfloat16
F8 = mybir.dt.float8e4
BF16_NP = ml_dtypes.bfloat16
E4_NP = ml_dtypes.float8_e4m3
AF = mybir.ActivationFunctionType
DR = mybir.MatmulPerfMode.DoubleRow

S = 2048          # sequence length (per batch)
E = 1024          # embedding dim
HL = 8            # heads per core
D = 64            # head dim
TT = S // 128     # 16 token tiles
NG = 4            # DoubleRow contraction groups of 256 over E
NCH = S // 1024   # 2 q-chunks of 1024
WS = 32.0          # weight pre-scale: q/k/v (and A^T, w_proj) carried 32x
EXP_SCALE = 0.125 / (WS * WS)
PRIO_OFFSET = 800  # attention body scheduled ahead of filler work
SCORES_FP8 = True


def _install_drain_fix():
    """walrus in this container rejects the Tile kernel-tail Drain when it
    carries all semaphore waits on one instruction ("Too many sync wait
    commands"). Emit one wait_ge per semaphore, then a bare drain."""
    if getattr(tile.TileContext, "_drain_fix_installed", False):
        return

    def _split_drain_and_barrier(self, tick_clock, wait_clock):
        nc = self.nc
        probe = mybir.InstDrain(
            name="probe-drain", engine=mybir.EngineType.SP, ins=[], outs=[]
        )
        wait_clock.add_sem_waits(probe, ScopedClock({None: tick_clock.global_clock}))
        waits = re.findall(r"wait:S\[([A-Za-z0-9_]+)\]>=(\d+)", probe.concise())
        handles = {h.name: h for h in self.sems.allocated().values()}
        for name, val in waits:
            nc.sync.wait_ge(handles[name], int(val))
        nc.sync.drain()
        nc.all_engine_barrier()
        popped = nc._tile_sem_poison_stack.pop()
        assert popped is self._sem_poison
        nc.clear_and_free_semaphores(list(self.sems.allocated().values()))
        nc.all_engine_barrier()

    tile.TileContext._drain_and_barrier = _split_drain_and_barrier
    tile.TileContext._drain_fix_installed = True


def _emit(nc, tc, ctx):
    xh_d = nc.declare_dram_parameter("xh", [128, NG, 2, S], F8, isOutput=False)
    xl_d = nc.declare_dram_parameter("xl", [128, NG, 2, S], F8, isOutput=False)
    wqh_d = nc.declare_dram_parameter("wqh", [128, NG, 2, 1024], F8, isOutput=False)
    wql_d = nc.declare_dram_parameter("wql", [128, NG, 2, 1024], F8, isOutput=False)
    wvh_d = nc.declare_dram_parameter("wvh", [128, NG, 2, 512], F8, isOutput=False)
    wvl_d = nc.declare_dram_parameter("wvl", [128, NG, 2, 512], F8, isOutput=False)
    wph_d = nc.declare_dram_parameter("wph", [128, 2, 2, E], F8, isOutput=False)
    wpl_d = nc.declare_dram_parameter("wpl", [128, 2, 2, E], F8, isOutput=False)
    bqk_d = nc.declare_dram_parameter("bqk", [8, 128, 1], F32, isOutput=False)
    bva_d = nc.declare_dram_parameter("bva", [1, 512], BF16, isOutput=False)
    out_d = nc.declare_dram_parameter("out", [S, E], BF16, isOutput=True)
    # tail-region (rows 1024:2048) c_proj partials, one per ct-PAIR;
    # summed on the host together with the core-pair reduction
    out2_d = nc.declare_dram_parameter("out2", [2, 1024, E], BF16, isOutput=True)

    consts = ctx.enter_context(tc.tile_pool(name="consts", bufs=1))
    statics = ctx.enter_context(tc.tile_pool(name="statics", bufs=1))
    ptp = ctx.enter_context(tc.tile_pool(name="ptp", bufs=6))
    rp = ctx.enter_context(tc.tile_pool(name="rp", bufs=2))
    yp = ctx.enter_context(tc.tile_pool(name="yp", bufs=3))
    # PSUM budget (8 banks):
    #   psS 1x[128,1024] = 2 (score tiles; exp-rate-bound so depth 1 is ok)
    #   psT 2x[128,128]  = 1 (A^T transpose staging, bf16)
    #   paP 2x[128,8,64] = 2 (reoriented attn@V accumulators, 1 bank each)
    #   dnP 1x[128,8]    = 1 (softmax denominators)
    #   psQ 2x[128,512]  = 2 (qkv / c_proj groups)
    psS = ctx.enter_context(tc.tile_pool(name="psS", bufs=1, space="PSUM"))
    psT = ctx.enter_context(tc.tile_pool(name="psT", bufs=1, space="PSUM"))
    paP = ctx.enter_context(tc.tile_pool(name="paP", bufs=2, space="PSUM"))
    dnP = ctx.enter_context(tc.tile_pool(name="dnP", bufs=1, space="PSUM"))
    psQ = ctx.enter_context(tc.tile_pool(name="psQ", bufs=2, space="PSUM"))

    # ---- front section: DMA order matters (the DMA engines are a single
    # serialized resource). The hh-term operands first so qkv starts early,
    # then the lo tensors, wva, and wp (needed last) at the end ----
    xh_sb = statics.tile([128, NG, 2, S], F8)
    xl_sb = statics.tile([128, NG, 2, S], F8)
    wqh_sb = statics.tile([128, NG, 2, 1024], F8)
    wql_sb = statics.tile([128, NG, 2, 1024], F8)
    wvh_sb = statics.tile([128, NG, 2, 512], F8)
    wvl_sb = statics.tile([128, NG, 2, 512], F8)
    wph_sb = statics.tile([128, 2, 2, E], F8)
    wpl_sb = statics.tile([128, 2, 2, E], F8)

    for g in range(NG):
        if g == 0:
            nc.sync.dma_start(out=xh_sb[:, 0, 0, :], in_=xh_d[:, 0, 0, :])
            nc.sync.dma_start(out=xh_sb[:, 0, 1, :], in_=xh_d[:, 0, 1, :])
        else:
            nc.sync.dma_start(out=xh_sb[:, g, :, :], in_=xh_d[:, g, :, :])
        nc.gpsimd.dma_start(out=wqh_sb[:, g, :, :], in_=wqh_d[:, g, :, :])
    for g in range(NG):
        nc.sync.dma_start(out=xl_sb[:, g, :, :], in_=xl_d[:, g, :, :])
        nc.gpsimd.dma_start(out=wql_sb[:, g, :, :], in_=wql_d[:, g, :, :])
    for g in range(NG):
        nc.gpsimd.dma_start(out=wvh_sb[:, g, :, :], in_=wvh_d[:, g, :, :])
        nc.gpsimd.dma_start(out=wvl_sb[:, g, :, :], in_=wvl_d[:, g, :, :])
    for g in range(2):
        nc.gpsimd.dma_start(out=wph_sb[:, g, :, :], in_=wph_d[:, g, :, :])
        nc.gpsimd.dma_start(out=wpl_sb[:, g, :, :], in_=wpl_d[:, g, :, :])

    bqk_sb = consts.tile([128, 8], F32)
    for m in range(8):
        nc.sync.dma_start(out=bqk_sb[:, m : m + 1], in_=bqk_d[m])
    bva_sb = consts.tile([1, 512], BF16)
    nc.sync.dma_start(out=bva_sb, in_=bva_d[:])
    ones_sb = consts.tile([1, 128], BF16)
    nc.gpsimd.memset(ones_sb[:], 1.0)
    # denominator column: value 1.0 makes pa*(1/den) carry 32*A (the V 32x
    # scale survives), which is exactly the e4m3 range A^T wants
    dcol_sb = consts.tile([128, 1], BF16)
    nc.gpsimd.memset(dcol_sb[:], 1.0)
    # identity for PE transposes
    ident_sb = consts.tile([128, 128], BF16)
    nc.gpsimd.memset(ident_sb[:], 1.0)
    nc.gpsimd.affine_select(
        out=ident_sb[:],
        in_=ident_sb[:],
        compare_op=mybir.AluOpType.is_equal,
        fill=0.0,
        base=0,
        pattern=[[1, 128]],
        channel_multiplier=-1,
    )

    # ---- qkv Q^T,K^T (W stationary, fp8 DR hi/lo x hi/lo, 32-scaled) paired
    # so head h's Q and K m-tiles arrive together, interleaved with V tiles ----
    qkt_sb = statics.tile([128, 8, S], F8)
    # one [64,...] tile per head pair: head h at partition 32*(h%2),
    # dim1 = Q/K, dim2 = the DoubleRow d-pair slot
    qk2_sb = [statics.tile([64, 2, 2, S], F8, name=f"qk2_{j}") for j in range(4)]
    va_sb = statics.tile([128, TT, 512], BF16)

    # split-product terms in DMA-arrival order: hh first, then hl, lh
    QK_TERMS = ((wqh_sb, xh_sb), (wql_sb, xh_sb), (wqh_sb, xl_sb))
    V_TERMS = ((wvh_sb, xh_sb), (wvl_sb, xh_sb), (wvh_sb, xl_sb))

    def emit_qk(m, ramp=False):
        # during the DMA-paced ramp the attention PSUM banks are still free:
        # spread the first pair's groups across them so more accumulations
        # are in flight per arriving weight tile
        pools = [psQ, psQ, paP, dnP] if ramp else [psQ] * 4
        tags = {id(psQ): "qk", id(paP): "o", id(dnP): "d"}
        for tch in range(4):
            pqk = pools[tch].tile([128, 512], F32, tag=tags[id(pools[tch])])
            n = 0
            for wsb, xsb in QK_TERMS:
                for g in range(NG):
                    for c in range(2):
                        nc.tensor.matmul(
                            pqk[:, c * 256 : (c + 1) * 256],
                            wsb[:, g, :, m * 128 : (m + 1) * 128],
                            xsb[:, g, :, tch * 512 + c * 256 : tch * 512 + (c + 1) * 256],
                            start=(n == 0),
                            stop=(n == 23),
                            perf_mode=DR,
                        )
                        n += 1
            nc.vector.tensor_scalar_add(
                qkt_sb[:, m, tch * 512 : (tch + 1) * 512], pqk, bqk_sb[:, m : m + 1]
            )

    def emit_regroup(h):
        # partition-regrouping DMAs for the [32,2] d-packed score operands
        po, pr = (h % 2) * 64, 32 * (h % 2)
        t = qk2_sb[h // 2]
        for m, g in ((h // 2, 0), (4 + h // 2, 1)):
            for i in (0, 1):
                nc.sync.dma_start(
                    out=t[pr : pr + 32, g, i, :],
                    in_=qkt_sb[po + 32 * i : po + 32 * i + 32, m, :],
                )

    def emit_v(i, ramp=False):
        pv1 = (psS if ramp else psQ).tile([128, 512], F32, tag="sc" if ramp else "qk")
        n = 0
        for wsb, xsb in V_TERMS:
            for g in range(NG):
                for c in range(2):
                    nc.tensor.matmul(
                        pv1[:, c * 256 : (c + 1) * 256],
                        xsb[:, g, :, i * 128 : (i + 1) * 128],
                        wsb[:, g, :, c * 256 : (c + 1) * 256],
                        start=(n == 0),
                        stop=False,
                        perf_mode=DR,
                    )
                    n += 1
        # V bias as a rank-1 ones-row matmul into the same PSUM group
        for c in range(2):
            nc.tensor.matmul(
                pv1[:, c * 256 : (c + 1) * 256],
                ones_sb[0:1, 0:128],
                bva_sb[0:1, c * 256 : (c + 1) * 256],
                start=False,
                stop=(c == 1),
            )
        nc.vector.tensor_copy(va_sb[:, i, :], pv1)

    # Q/K pairs 0-1 and V tiles 0-7 up front (enough for chunk-0 heads 0-3);
    # pairs 2-3 are deferred into the chunk-0 head loop as PE filler, arriving
    # three heads before their consumers
    for m in range(2):
        emit_qk(m, ramp=True)      # Q m-tile: heads 2m, 2m+1
        emit_qk(4 + m, ramp=True)  # K m-tile: heads 2m, 2m+1
        if SCORES_FP8:
            emit_regroup(2 * m)
            emit_regroup(2 * m + 1)
        emit_v(4 * m, ramp=True)
        emit_v(4 * m + 1, ramp=True)
        emit_v(4 * m + 2)
        emit_v(4 * m + 3)

    # ---- attention (q-chunks of 1024), interleaved with c_proj halves ----
    # A^T (32-scaled): bf16 master + e4m3 hi/lo pair for the DR c_proj
    at32_sb = statics.tile([128, 4, S], BF16)
    ath_sb = statics.tile([128, 4, S], F8)
    atl_sb = statics.tile([128, 4, S], F8)
    # [q, c]-oriented normalized attn out per head pair (double-buffered)
    asb_sb = statics.tile([128, 2, 8, 128], BF16)

    CP_TERMS = ((ath_sb, wph_sb), (ath_sb, wpl_sb), (atl_sb, wph_sb))

    def emit_cproj(i):
        ysb = yp.tile([128, E], BF16, tag="y")
        for ech in range(2):
            py = psQ.tile([128, 512], F32, tag="qk")
            n = 0
            for asb, wsb in CP_TERMS:
                for g in range(2):
                    for c in range(2):
                        nc.tensor.matmul(
                            py[:, c * 256 : (c + 1) * 256],
                            asb[:, 2 * g : 2 * g + 2, i * 128 : (i + 1) * 128],
                            wsb[:, g, :, ech * 512 + c * 256 : ech * 512 + (c + 1) * 256],
                            start=(n == 0),
                            stop=(n == 11),
                            perf_mode=DR,
                        )
                        n += 1
            nc.vector.tensor_copy(ysb[:, ech * 512 : (ech + 1) * 512], py)
            nc.sync.dma_start(
                out=out_d[i * 128 : (i + 1) * 128, ech * 512 : (ech + 1) * 512],
                in_=ysb[:, ech * 512 : (ech + 1) * 512],
            )

    def emit_cproj_tail(ctp):
        # tail-region (q rows 1024:2048) partial for ct-pair ctp: contraction
        # is one DR group (256 rows), host sums the 2 partials
        for i in range(8, 16):
            y2 = yp.tile([128, E], BF16, tag="y")
            for ech in range(2):
                py = psQ.tile([128, 512], F32, tag="qk")
                n = 0
                for asb, wsb in CP_TERMS:
                    for c in range(2):
                        nc.tensor.matmul(
                            py[:, c * 256 : (c + 1) * 256],
                            asb[:, 2 * ctp : 2 * ctp + 2, i * 128 : (i + 1) * 128],
                            wsb[:, ctp, :, ech * 512 + c * 256 : ech * 512 + (c + 1) * 256],
                            start=(n == 0),
                            stop=(n == 5),
                            perf_mode=DR,
                        )
                        n += 1
                nc.vector.tensor_copy(y2[:, ech * 512 : (ech + 1) * 512], py)
            nc.sync.dma_start(
                out=out2_d[ctp, (i - 8) * 128 : (i - 7) * 128, :], in_=y2
            )

    for j in range(NCH):
        q0 = j * 1024
        nkt = 8 * (j + 1)
        for h in range(HL):
            po = (h % 2) * 64
            qm, km = h // 2, 4 + h // 2
            ctx_hp = tc.high_priority(offset=PRIO_OFFSET)
            ctx_hp.__enter__()
            # one bank holds all 8 q-tile accumulators [128, qc, 64];
            # denominators accumulate per q-tile column in dnP
            pa = paP.tile([128, 8, 64], F32, tag="o")
            den = dnP.tile([128, 8], F32, tag="d")
            for kt in range(nkt):
                p = kt - 8 * j
                off = max(0, p * 128)
                qc0 = off // 128
                ps2 = psS.tile([128, 1024], F32, tag="sc")
                if SCORES_FP8:
                    t, pr = qk2_sb[h // 2], 32 * (h % 2)
                    aa = off
                    while aa < 1024:
                        bb = min(1024, (aa // 256 + 1) * 256)
                        nc.tensor.matmul(
                            ps2[:, aa:bb],
                            t[pr : pr + 32, 1, :, kt * 128 : (kt + 1) * 128],
                            t[pr : pr + 32, 0, :, q0 + aa : q0 + bb],
                            start=True,
                            stop=True,
                            perf_mode=DR,
                        )
                        aa = bb
                else:
                    for a, b in ([(off, 512), (512, 1024)] if off < 512 else [(off, 1024)]):
                        nc.tensor.matmul(
                            ps2[:, a:b],
                            qkt_sb[po : po + 64, km, kt * 128 : (kt + 1) * 128],
                            qkt_sb[po : po + 64, qm, q0 + a : q0 + b],
                            start=True,
                            stop=True,
                        )
                pt = ptp.tile([128, 1024], BF16, tag="pt")
                nc.scalar.activation(
                    out=pt[:, off:1024], in_=ps2[:, off:1024], func=AF.Exp,
                    scale=EXP_SCALE,
                )
                if p >= 0:
                    # causal triangle on the diagonal 128-block: keep where
                    # q >= k, zero elsewhere (Pool engine; ACT is exp-bound)
                    nc.gpsimd.affine_select(
                        out=pt[:, off : off + 128],
                        in_=pt[:, off : off + 128],
                        compare_op=mybir.AluOpType.is_ge,
                        fill=0.0,
                        base=0,
                        pattern=[[1, 128]],
                        channel_multiplier=-1,
                    )
                # reoriented attn@V: out[q, d] — P stationary, V moving
                # (64 cols), one matmul per live q-tile; denominator via a
                # rank-1 ones column into the shared den bank
                for qc in range(qc0, 8):
                    qs = qc * 128
                    nc.tensor.matmul(
                        pa[:, qc, :],
                        pt[:, qs : qs + 128],
                        va_sb[:, kt, h * 64 : (h + 1) * 64],
                        start=(kt == 0 and qc == 0),
                        stop=(kt == 8 * j + qc),
                        skip_group_check=True,
                    )
                    nc.tensor.matmul(
                        den[:, qc : qc + 1],
                        pt[:, qs : qs + 128],
                        dcol_sb[:, :],
                        start=(kt == 0 and qc == 0),
                        stop=(kt == 8 * j + qc),
                        skip_group_check=True,
                    )
            rinv = rp.tile([128, 8], F32, tag="ri")
            nc.vector.reciprocal(out=rinv, in_=den)
            # normalization folded into the PSUM drain: asb[q, c] = pa * rinv
            # (per-partition scalar). GPSIMD can't touch PSUM, so DVE only.
            for qc in range(8):
                nc.vector.tensor_scalar_mul(
                    asb_sb[:, (h // 2) % 2, qc, po : po + 64],
                    pa[:, qc, :],
                    rinv[:, qc : qc + 1],
                )
            ctx_hp.__exit__(None, None, None)
            if h % 2 == 1:
                # rebuild A^T for the completed head pair: PE transpose per
                # q-tile, DVE drains the bf16 master, Pool (SBUF-only) derives
                # the e4m3 hi/lo pair c_proj consumes
                hp = h // 2
                ctx_hp = tc.high_priority(offset=PRIO_OFFSET)
                ctx_hp.__enter__()
                for qc in range(8):
                    qs = q0 + qc * 128
                    pst = psT.tile([128, 128], BF16, tag="t")
                    nc.tensor.transpose(pst, asb_sb[:, hp % 2, qc, :], ident_sb)
                    nc.vector.tensor_copy(at32_sb[:, hp, qs : qs + 128], pst)
                    nc.gpsimd.tensor_copy(
                        ath_sb[:, hp, qs : qs + 128], at32_sb[:, hp, qs : qs + 128]
                    )
                    nc.gpsimd.tensor_sub(
                        atl_sb[:, hp, qs : qs + 128],
                        at32_sb[:, hp, qs : qs + 128],
                        ath_sb[:, hp, qs : qs + 128],
                    )
                ctx_hp.__exit__(None, None, None)
            if j == 0:
                # V tiles 8-15 (needed only by chunk 1) as PE filler while
                # chunk-0 attention is ACT(exp)-rate-bound
                emit_v(8 + h)
                if h == 1:
                    emit_qk(2)
                    emit_qk(6)
                    if SCORES_FP8:
                        emit_regroup(4)
                        emit_regroup(5)
                elif h == 3:
                    emit_qk(3)
                    emit_qk(7)
                    if SCORES_FP8:
                        emit_regroup(6)
                        emit_regroup(7)
            else:
                # chunk-0 c_proj tiles as PE filler for chunk-1 attention
                emit_cproj(h)
                if h == 3:
                    emit_cproj_tail(0)
                elif h == 7:
                    emit_cproj_tail(1)


def build_nc():
    _install_drain_fix()
    from contextlib import ExitStack

    nc = bacc.Bacc()
    with ExitStack() as ctx:
        tc = ctx.enter_context(tile.TileContext(nc))
        _emit(nc, tc, ctx)
    nc.finalize()  # Bacc: alloc_regs + insert_library_loads happen here
    return nc


def _split_pack(a, scale, ng):
    """Split f32 array [rows, cols] into e4m3 (hi, lo) of scale*a, packed
    [128, ng, 2, cols] so DR group g, slot i, partition p holds row
    256*g + 128*i + p."""
    a = np.asarray(a, dtype=np.float32) * scale
    hi = a.astype(E4_NP)
    lo = (a - hi.astype(np.float32)).astype(E4_NP)
    cols = a.shape[1]

    def pack(x):
        return np.ascontiguousarray(
            x.reshape(ng, 2, 128, cols).transpose(2, 0, 1, 3)
        )

    return pack(hi), pack(lo)


def make_in_maps(inputs, w_attn, b_attn, w_proj, b_proj):
    """Build the 8 per-core input dicts from the full tensors."""
    x = np.asarray(inputs, dtype=np.float32)
    w_attn = np.asarray(w_attn, dtype=np.float32)
    b_attn = np.asarray(b_attn, dtype=np.float32)
    w_proj = np.asarray(w_proj, dtype=np.float32)

    # X^T splits are per batch (shared by the core pair)
    xsp = [_split_pack(x[b].T, 1.0, NG) for b in range(4)]

    in_maps = []
    for c in range(8):
        b, half = c // 2, c % 2
        h0 = half * 8
        cols = np.arange(h0 * 64, h0 * 64 + 512)
        wqk = np.concatenate([w_attn[:, cols], w_attn[:, 1024 + cols]], axis=1)
        wqh, wql = _split_pack(wqk, WS, NG)
        bqk = (WS * np.concatenate([b_attn[cols], b_attn[1024 + cols]])).reshape(
            8, 128, 1
        )
        vbase = 2048 + h0 * 64
        wvh, wvl = _split_pack(w_attn[:, vbase : vbase + 512], WS, NG)
        bva = (WS * b_attn[vbase : vbase + 512]).reshape(1, 512)
        wph, wpl = _split_pack(w_proj[h0 * 64 : h0 * 64 + 512, :], WS, 2)
        in_maps.append(
            {
                "xh": xsp[b][0],
                "xl": xsp[b][1],
                "wqh": wqh,
                "wql": wql,
                "wvh": wvh,
                "wvl": wvl,
                "wph": wph,
                "wpl": wpl,
                "bqk": np.ascontiguousarray(bqk.astype(np.float32)),
                "bva": np.ascontiguousarray(bva.astype(BF16_NP)),
            }
        )
    return in_maps


_CACHE = {}


def kernel(**inputs):
    nc = _CACHE.get("nc")
    if nc is None:
        nc = _CACHE["nc"] = build_nc()
    in_maps = make_in_maps(
        inputs["inputs"],
        inputs["w_attn"],
        inputs["b_attn"],
        inputs["w_proj"],
        inputs["b_proj"],
    )
    res = run_bass_kernel_spmd(nc, in_maps, core_ids=list(range(8)))
    return gather(res.results, inputs["b_proj"])


def gather(results, b_proj):
    # device output carries the (32*A)·(32*Wp) = 1024x weight scale
    out = np.zeros((4, S, E), dtype=np.float32)
    for b in range(4):
        for c in (2 * b, 2 * b + 1):
            r = results[c]
            # rows 0:1024 come from "out"; the device writes rows 1024:2048
            # only via the per-ct-pair partials in "out2"
            out[b, 0:1024] += r["out"][0:1024].astype(np.float32)
            out[b, 1024:2048] += r["out2"].astype(np.float32).sum(axis=0)
    out *= 1.0 / (WS * WS)
    out += np.asarray(b_proj, dtype=np.float32)[None, None, :]
    return out


# revision 55
# speedup vs baseline: 1.2855x; 1.2855x over previous
"""GPT-2 style causal attention block (B=4, S=2048, E=1024, H=16, D=64) on
8 TRN2 NeuronCores.

Sharding: batch(4) x head-half(2) -> 8 cores, zero on-device communication.
Core c handles batch b=c//2 and heads h0=(c%2)*8 .. h0+7. Each core computes
its qkv column block, attention for its 8 heads, and a partial c_proj
(its 512 rows of w_proj). The partial outputs per batch are summed on the
host during unshard (which also applies the 1/1024 weight-scale and b_proj).

fp8 DoubleRow usage (error-free hi/lo residual splits unless noted):
- qkv: X^T and weights pre-split on the host into e4m3 (hi, lo) packed
  [128, 4, 2, cols]; each DR matmul contracts 256 embedding rows at 0.5
  cyc/col; hh+hl+lh gives 6 column-passes vs bf16's 8. Weights are 32x
  pre-scaled (e4m3 normal range), so Q^T/K^T/V are carried 32-scaled.
- scores: Q^T/K^T quantized to e4m3 (plain, ~1.1e-2 added rel err) and
  DMA-regrouped so head-dim contracts as a DR [32, 2] pack at 0.5 cyc/col.
- c_proj: A^T carried as a 32-scaled e4m3 (hi, lo) pair, w_proj 32x
  pre-scaled and split on the host; ct-pairs contract 256 rows per DR
  matmul (3 split terms = 6 passes vs bf16's 8). The 1/1024 descale and
  b_proj land in the host-side gather.

Attention (per head, q-chunks of 1024): scores^T[k, q] via W-stationary
matmuls, exp on ACT (the dominant ACT cost, ~135us: it bounds how much
the other engines may carry), causality by computing only k<=q 128-tiles
plus a gpsimd affine_select on each diagonal 128-block. attn@V is
REORIENTED: out[q, d] per (q-tile, kt) with P as the stationary operand
pays 64 columns instead of 128 -> half the PE cost of the [d, q] form.
The 8 q-tile accumulators of a chunk pack into ONE PSUM bank [128, 8, 64];
softmax denominators accumulate via rank-1 ones-column matmuls into a
second bank, giving one batched reciprocal per (head, chunk) and a
normalization that is folded into the PSUM-drain copy (per-partition
scalar). A^T is then rebuilt per head-pair by PE transposes (identity
matmul) with psum drains split across DVE/Pool, writing the e4m3 hi/lo
pair that c_proj consumes.

Scheduling: attention bodies priority-boosted over filler (qkv pairs 2-3,
V tiles 8-15, c_proj tiles) which is interleaved into the exp-bound
stretches; during the DMA-paced ramp the qkv groups borrow the idle
attention PSUM banks. PSUM accumulates f32; copies avoid ACT entirely
(exp saturates it) and alternate DVE/Pool.
"""

import re

import ml_dtypes
import numpy as np

import concourse.mybir as mybir
import concourse.tile as tile
from concourse import bacc
from concourse.bass_utils import run_bass_kernel_spmd
from concourse.vector_clock import ScopedClock

F32 = mybir.dt.float32
BF16 = mybir.dt.bfloat16
F8 = mybir.dt.float8e4
BF16_NP = ml_dtypes.bfloat16
E4_NP = ml_dtypes.float8_e4m3
AF = mybir.ActivationFunctionType
DR = mybir.MatmulPerfMode.DoubleRow

S = 2048          # sequence length (per batch)
E = 1024          # embedding dim
HL = 8            # heads per core
D = 64            # head dim
TT = S // 128     # 16 token tiles
NG = 4            # DoubleRow contraction groups of 256 over E
NCH = S // 1024   # 2 q-chunks of 1024
WS = 32.0          # weight pre-scale: q/k/v (and A^T, w_proj) carried 32x
EXP_SCALE = 0.125 / (WS * WS)
PRIO_OFFSET = 800  # attention body scheduled ahead of filler work
SCORES_FP8 = True


def _install_drain_fix():
    """walrus in this container rejects the Tile kernel-tail Drain when it
    carries all semaphore waits on one instruction ("Too many sync wait
    commands"). Emit one wait_ge per semaphore, then a bare drain."""
    if getattr(tile.TileContext, "_drain_fix_installed", False):
        return

    def _split_drain_and_barrier(self, tick_clock, wait_clock):
        nc = self.nc
        probe = mybir.InstDrain(
            name="probe-drain", engine=mybir.EngineType.SP, ins=[], outs=[]
        )
        wait_clock.add_sem_waits(probe, ScopedClock({None: tick_clock.global_clock}))
        waits = re.findall(r"wait:S\[([A-Za-z0-9_]+)\]>=(\d+)", probe.concise())
        handles = {h.name: h for h in self.sems.allocated().values()}
        for name, val in waits:
            nc.sync.wait_ge(handles[name], int(val))
        nc.sync.drain()
        nc.all_engine_barrier()
        popped = nc._tile_sem_poison_stack.pop()
        assert popped is self._sem_poison
        nc.clear_and_free_semaphores(list(self.sems.allocated().values()))
        nc.all_engine_barrier()

    tile.TileContext._drain_and_barrier = _split_drain_and_barrier
    tile.TileContext._drain_fix_installed = True


def _emit(nc, tc, ctx):
    xh_d = nc.declare_dram_parameter("xh", [128, NG, 2, S], F8, isOutput=False)
    xl_d = nc.declare_dram_parameter("xl", [128, NG, 2, S], F8, isOutput=False)
    wqh_d = nc.declare_dram_parameter("wqh", [128, NG, 2, 1024], F8, isOutput=False)
    wql_d = nc.declare_dram_parameter("wql", [128, NG, 2, 1024], F8, isOutput=False)
    wvh_d = nc.declare_dram_parameter("wvh", [128, NG, 2, 512], F8, isOutput=False)
    wvl_d = nc.declare_dram_parameter("wvl", [128, NG, 2, 512], F8, isOutput=False)
    wph_d = nc.declare_dram_parameter("wph", [128, 2, 2, E], F8, isOutput=False)
    wpl_d = nc.declare_dram_parameter("wpl", [128, 2, 2, E], F8, isOutput=False)
    bqk_d = nc.declare_dram_parameter("bqk", [128, 8], F32, isOutput=False)
    bva_d = nc.declare_dram_parameter("bva", [1, 512], BF16, isOutput=False)
    out_d = nc.declare_dram_parameter("out", [S, E], BF16, isOutput=True)
    # tail-region (rows 1024:2048) c_proj partials, one per ct-PAIR;
    # summed on the host together with the core-pair reduction
    out2_d = nc.declare_dram_parameter("out2", [2, 1024, E], BF16, isOutput=True)

    consts = ctx.enter_context(tc.tile_pool(name="consts", bufs=1))
    statics = ctx.enter_context(tc.tile_pool(name="statics", bufs=1))
    ptp = ctx.enter_context(tc.tile_pool(name="ptp", bufs=8))
    rp = ctx.enter_context(tc.tile_pool(name="rp", bufs=2))
    yp = ctx.enter_context(tc.tile_pool(name="yp", bufs=3))
    # PSUM budget (8 banks):
    #   psS 2x[128,1024] = 4 (score tiles: depth 2 so scores(kt+1) overlaps
    #                         exp(kt) — the ACT exp stream must never starve)
    #   paP 2x[128,8,64] = 2 (reoriented attn@V accumulators, 1 bank each)
    #   dnP 1x[128,8]    = 1 (softmax denominators)
    #   psQ 1x[128,512]  = 1 (qkv / c_proj groups + A^T transpose staging)
    psS = ctx.enter_context(tc.tile_pool(name="psS", bufs=2, space="PSUM"))
    paP = ctx.enter_context(tc.tile_pool(name="paP", bufs=2, space="PSUM"))
    dnP = ctx.enter_context(tc.tile_pool(name="dnP", bufs=1, space="PSUM"))
    psQ = ctx.enter_context(tc.tile_pool(name="psQ", bufs=1, space="PSUM"))

    # ---- front section: DMA order matters (the DMA engines are a single
    # serialized resource). The hh-term operands first so qkv starts early,
    # then the lo tensors, wva, and wp (needed last) at the end ----
    xh_sb = statics.tile([128, NG, 2, S], F8)
    xl_sb = statics.tile([128, NG, 2, S], F8)
    wqh_sb = statics.tile([128, NG, 2, 1024], F8)
    wql_sb = statics.tile([128, NG, 2, 1024], F8)
    wvh_sb = statics.tile([128, NG, 2, 512], F8)
    wvl_sb = statics.tile([128, NG, 2, 512], F8)
    wph_sb = statics.tile([128, 2, 2, E], F8)
    wpl_sb = statics.tile([128, 2, 2, E], F8)

    for g in range(NG):
        nc.gpsimd.dma_start(out=wqh_sb[:, g, :, :], in_=wqh_d[:, g, :, :])
        if g == 0:
            nc.sync.dma_start(out=xh_sb[:, 0, 0, :], in_=xh_d[:, 0, 0, :])
            nc.sync.dma_start(out=xh_sb[:, 0, 1, :], in_=xh_d[:, 0, 1, :])
        else:
            nc.sync.dma_start(out=xh_sb[:, g, :, :], in_=xh_d[:, g, :, :])
    for g in range(NG):
        nc.gpsimd.dma_start(out=wql_sb[:, g, :, :], in_=wql_d[:, g, :, :])
    # xl rides both queues so the lh split-terms unlock ~3us earlier
    for g in range(NG):
        (nc.sync if g < 2 else nc.gpsimd).dma_start(
            out=xl_sb[:, g, :, :], in_=xl_d[:, g, :, :]
        )
    for g in range(NG):
        nc.gpsimd.dma_start(out=wvh_sb[:, g, :, :], in_=wvh_d[:, g, :, :])
        nc.gpsimd.dma_start(out=wvl_sb[:, g, :, :], in_=wvl_d[:, g, :, :])
    for g in range(2):
        nc.gpsimd.dma_start(out=wph_sb[:, g, :, :], in_=wph_d[:, g, :, :])
        nc.gpsimd.dma_start(out=wpl_sb[:, g, :, :], in_=wpl_d[:, g, :, :])

    bqk_sb = consts.tile([128, 8], F32)
    nc.sync.dma_start(out=bqk_sb[:, :], in_=bqk_d[:, :])
    bva_sb = consts.tile([1, 512], BF16)
    nc.sync.dma_start(out=bva_sb, in_=bva_d[:])
    ones_sb = consts.tile([1, 128], BF16)
    nc.gpsimd.memset(ones_sb[:], 1.0)
    # denominator column: value 1.0 makes pa*(1/den) carry 32*A (the V 32x
    # scale survives), which is exactly the e4m3 range A^T wants
    dcol_sb = consts.tile([128, 1], BF16)
    nc.gpsimd.memset(dcol_sb[:], 1.0)
    # identity for PE transposes
    ident_sb = consts.tile([128, 128], BF16)
    nc.gpsimd.memset(ident_sb[:], 1.0)
    nc.gpsimd.affine_select(
        out=ident_sb[:],
        in_=ident_sb[:],
        compare_op=mybir.AluOpType.is_equal,
        fill=0.0,
        base=0,
        pattern=[[1, 128]],
        channel_multiplier=-1,
    )

    # ---- qkv Q^T,K^T (W stationary, fp8 DR hi/lo x hi/lo, 32-scaled) paired
    # so head h's Q and K m-tiles arrive together, interleaved with V tiles ----
    qkt_sb = statics.tile([128, 8, S], F8)
    # one [64,...] tile per head pair: head h at partition 32*(h%2),
    # dim1 = Q/K, dim2 = the DoubleRow d-pair slot
    qk2_sb = [statics.tile([64, 2, 2, S], F8, name=f"qk2_{j}") for j in range(4)]
    va_sb = statics.tile([128, TT, 512], BF16)

    # split-product terms in DMA-arrival order: hh first, then hl, lh
    QK_TERMS = ((wqh_sb, xh_sb), (wql_sb, xh_sb), (wqh_sb, xl_sb))
    V_TERMS = ((wvh_sb, xh_sb), (wvl_sb, xh_sb), (wvh_sb, xl_sb))

    def emit_qk(m, ramp=False):
        # during the DMA-paced ramp the attention PSUM banks are still free:
        # spread the first pair's groups across them so more accumulations
        # are in flight per arriving weight tile
        if ramp == "attn":
            pools = [psQ, paP, paP, dnP]
        elif ramp:
            pools = [psS, psS, psS, psS]
        else:
            pools = [psQ] * 4
        tags = {id(psQ): "qk", id(paP): "o", id(dnP): "d", id(psS): "sc"}
        for tch in range(4):
            pqk = pools[tch].tile([128, 512], F32, tag=tags[id(pools[tch])])
            n = 0
            for wsb, xsb in QK_TERMS:
                for g in range(NG):
                    for c in range(2):
                        nc.tensor.matmul(
                            pqk[:, c * 256 : (c + 1) * 256],
                            wsb[:, g, :, m * 128 : (m + 1) * 128],
                            xsb[:, g, :, tch * 512 + c * 256 : tch * 512 + (c + 1) * 256],
                            start=(n == 0),
                            stop=(n == 23),
                            perf_mode=DR,
                        )
                        n += 1
            nc.vector.tensor_scalar_add(
                qkt_sb[:, m, tch * 512 : (tch + 1) * 512], pqk, bqk_sb[:, m : m + 1]
            )

    def emit_regroup(h):
        # partition-regrouping DMAs for the [32,2] d-packed score operands
        po, pr = (h % 2) * 64, 32 * (h % 2)
        t = qk2_sb[h // 2]
        for m, g in ((h // 2, 0), (4 + h // 2, 1)):
            for i in (0, 1):
                nc.sync.dma_start(
                    out=t[pr : pr + 32, g, i, :],
                    in_=qkt_sb[po + 32 * i : po + 32 * i + 32, m, :],
                )

    def emit_v(i, pool=None):
        # never the psS pool: its buf rotation would gate the first score
        # tiles behind the (wv-DMA-gated) V fills
        pool = pool or psQ
        tag = {id(psQ): "qk", id(paP): "o", id(dnP): "d", id(psS): "sc"}[id(pool)]
        pv1 = pool.tile([128, 512], F32, tag=tag)
        n = 0
        for wsb, xsb in V_TERMS:
            for g in range(NG):
                for c in range(2):
                    nc.tensor.matmul(
                        pv1[:, c * 256 : (c + 1) * 256],
                        xsb[:, g, :, i * 128 : (i + 1) * 128],
                        wsb[:, g, :, c * 256 : (c + 1) * 256],
                        start=(n == 0),
                        stop=False,
                        perf_mode=DR,
                    )
                    n += 1
        # V bias as a rank-1 ones-row matmul into the same PSUM group
        for c in range(2):
            nc.tensor.matmul(
                pv1[:, c * 256 : (c + 1) * 256],
                ones_sb[0:1, 0:128],
                bva_sb[0:1, c * 256 : (c + 1) * 256],
                start=False,
                stop=(c == 1),
            )
        nc.vector.tensor_copy(va_sb[:, i, :], pv1)

    # Q/K pairs 0-2 and V tiles 0-7 up front (the hh/hl split terms fill the
    # early-DMA window); pair 3 is deferred into the chunk-0 head loop as PE
    # filler. V0/V1 borrow the score banks (drained before the first score
    # tile's turn in the rotation); V2-7 stream through psQ and may lag —
    # attn@V catches up behind the exp stream.
    emit_qk(0, ramp="attn")    # pair 0 borrows the attention banks
    emit_qk(4, ramp="attn")
    if SCORES_FP8:
        emit_regroup(0)
        emit_regroup(1)
    emit_qk(1, ramp=True)      # pairs 1-2 borrow the score banks
    emit_qk(5, ramp=True)
    if SCORES_FP8:
        emit_regroup(2)
        emit_regroup(3)
    emit_qk(2, ramp=True)
    emit_qk(6, ramp=True)
    if SCORES_FP8:
        emit_regroup(4)
        emit_regroup(5)
    for i in range(8):
        emit_v(i)

    # ---- attention (q-chunks of 1024), interleaved with c_proj halves ----
    # A^T (32-scaled): bf16 master + e4m3 hi/lo pair for the DR c_proj
    at32_sb = statics.tile([128, 4, S], BF16)
    ath_sb = statics.tile([128, 4, S], F8)
    atl_sb = statics.tile([128, 4, S], F8)
    # [q, c]-oriented normalized attn out per head pair (double-buffered)
    asb_sb = statics.tile([128, 2, 8, 128], BF16)

    CP_TERMS = ((ath_sb, wph_sb), (ath_sb, wpl_sb), (atl_sb, wph_sb))

    def emit_cproj(i):
        ysb = yp.tile([128, E], BF16, tag="y")
        for ech in range(2):
            py = psQ.tile([128, 512], F32, tag="qk")
            n = 0
            for asb, wsb in CP_TERMS:
                for g in range(2):
                    for c in range(2):
                        nc.tensor.matmul(
                            py[:, c * 256 : (c + 1) * 256],
                            asb[:, 2 * g : 2 * g + 2, i * 128 : (i + 1) * 128],
                            wsb[:, g, :, ech * 512 + c * 256 : ech * 512 + (c + 1) * 256],
                            start=(n == 0),
                            stop=(n == 11),
                            perf_mode=DR,
                        )
                        n += 1
            nc.vector.tensor_copy(ysb[:, ech * 512 : (ech + 1) * 512], py)
            nc.sync.dma_start(
                out=out_d[i * 128 : (i + 1) * 128, ech * 512 : (ech + 1) * 512],
                in_=ysb[:, ech * 512 : (ech + 1) * 512],
            )

    def emit_cproj_tail(ctp, last=False):
        # tail-region (q rows 1024:2048) partial for ct-pair ctp: contraction
        # is one DR group (256 rows), host sums the 2 partials. The final
        # tail is a pure epilogue: attention banks (paP) are free by then and
        # ACT is exp-idle, so spread psum groups and drains across both.
        for i in range(8, 16):
            y2 = yp.tile([128, E], BF16, tag="y")
            for ech in range(2):
                if last:
                    py = paP.tile([128, 512], F32, tag="o")
                else:
                    py = psQ.tile([128, 512], F32, tag="qk")
                n = 0
                for asb, wsb in CP_TERMS:
                    for c in range(2):
                        nc.tensor.matmul(
                            py[:, c * 256 : (c + 1) * 256],
                            asb[:, 2 * ctp : 2 * ctp + 2, i * 128 : (i + 1) * 128],
                            wsb[:, ctp, :, ech * 512 + c * 256 : ech * 512 + (c + 1) * 256],
                            start=(n == 0),
                            stop=(n == 5),
                            perf_mode=DR,
                        )
                        n += 1
                if last and (i + ech) % 2 == 0:
                    nc.scalar.copy(out=y2[:, ech * 512 : (ech + 1) * 512], in_=py)
                else:
                    nc.vector.tensor_copy(y2[:, ech * 512 : (ech + 1) * 512], py)
            nc.sync.dma_start(
                out=out2_d[ctp, (i - 8) * 128 : (i - 7) * 128, :], in_=y2
            )

    for j in range(NCH):
        q0 = j * 1024
        nkt = 8 * (j + 1)
        for h in range(HL):
            po = (h % 2) * 64
            qm, km = h // 2, 4 + h // 2
            ctx_hp = tc.high_priority(offset=PRIO_OFFSET)
            ctx_hp.__enter__()
            # one bank holds all 8 q-tile accumulators [128, qc, 64];
            # denominators accumulate per q-tile column in dnP
            pa = paP.tile([128, 8, 64], F32, tag="o")
            den = dnP.tile([128, 8], F32, tag="d")
            for kt in range(nkt):
                p = kt - 8 * j
                off = max(0, p * 128)
                qc0 = off // 128
                ps2 = psS.tile([128, 1024], F32, tag="sc")
                if SCORES_FP8:
                    t, pr = qk2_sb[h // 2], 32 * (h % 2)
                    aa = off
                    while aa < 1024:
                        bb = min(1024, (aa // 256 + 1) * 256)
                        nc.tensor.matmul(
                            ps2[:, aa:bb],
                            t[pr : pr + 32, 1, :, kt * 128 : (kt + 1) * 128],
                            t[pr : pr + 32, 0, :, q0 + aa : q0 + bb],
                            start=True,
                            stop=True,
                            perf_mode=DR,
                        )
                        aa = bb
                else:
                    for a, b in ([(off, 512), (512, 1024)] if off < 512 else [(off, 1024)]):
                        nc.tensor.matmul(
                            ps2[:, a:b],
                            qkt_sb[po : po + 64, km, kt * 128 : (kt + 1) * 128],
                            qkt_sb[po : po + 64, qm, q0 + a : q0 + b],
                            start=True,
                            stop=True,
                        )
                pt = ptp.tile([128, 1024], BF16, tag="pt")
                nc.scalar.activation(
                    out=pt[:, off:1024], in_=ps2[:, off:1024], func=AF.Exp,
                    scale=EXP_SCALE,
                )
                if p >= 0:
                    # causal triangle on the diagonal 128-block: keep where
                    # q >= k, zero elsewhere (Pool engine; ACT is exp-bound)
                    nc.gpsimd.affine_select(
                        out=pt[:, off : off + 128],
                        in_=pt[:, off : off + 128],
                        compare_op=mybir.AluOpType.is_ge,
                        fill=0.0,
                        base=0,
                        pattern=[[1, 128]],
                        channel_multiplier=-1,
                    )
                # reoriented attn@V: out[q, d] — P stationary, V moving
                # (64 cols), one matmul per live q-tile; denominator via a
                # rank-1 ones column into the shared den bank
                for qc in range(qc0, 8):
                    qs = qc * 128
                    nc.tensor.matmul(
                        pa[:, qc, :],
                        pt[:, qs : qs + 128],
                        va_sb[:, kt, h * 64 : (h + 1) * 64],
                        start=(kt == 0 and qc == 0),
                        stop=(kt == 8 * j + qc),
                        skip_group_check=True,
                    )
                    nc.tensor.matmul(
                        den[:, qc : qc + 1],
                        pt[:, qs : qs + 128],
                        dcol_sb[:, :],
                        start=(kt == 0 and qc == 0),
                        stop=(kt == 8 * j + qc),
                        skip_group_check=True,
                    )
            rinv = rp.tile([128, 8], F32, tag="ri")
            nc.vector.reciprocal(out=rinv, in_=den)
            # normalization folded into the PSUM drain: asb[q, c] = pa * rinv
            # (per-partition scalar). GPSIMD can't touch PSUM, so DVE only.
            for qc in range(8):
                nc.vector.tensor_scalar_mul(
                    asb_sb[:, (h // 2) % 2, qc, po : po + 64],
                    pa[:, qc, :],
                    rinv[:, qc : qc + 1],
                )
            ctx_hp.__exit__(None, None, None)
            if h % 2 == 1:
                # rebuild A^T for the completed head pair: PE transpose per
                # q-tile, DVE drains the bf16 master, Pool (SBUF-only) derives
                # the e4m3 hi/lo pair c_proj consumes
                hp = h // 2
                ctx_hp = tc.high_priority(offset=PRIO_OFFSET)
                ctx_hp.__enter__()
                for qc in range(8):
                    qs = q0 + qc * 128
                    pst = dnP.tile([128, 128], BF16, tag="d")
                    nc.tensor.transpose(pst, asb_sb[:, hp % 2, qc, :], ident_sb)
                    nc.vector.tensor_copy(at32_sb[:, hp, qs : qs + 128], pst)
                    nc.gpsimd.tensor_copy(
                        ath_sb[:, hp, qs : qs + 128], at32_sb[:, hp, qs : qs + 128]
                    )
                    nc.gpsimd.tensor_sub(
                        atl_sb[:, hp, qs : qs + 128],
                        at32_sb[:, hp, qs : qs + 128],
                        ath_sb[:, hp, qs : qs + 128],
                    )
                ctx_hp.__exit__(None, None, None)
            if j == 0:
                # deferred qkv as PE filler, spread so heads 6-7's Q/K (and
                # chunk 1's V tiles 8-15) are ready well before their readers
                if h == 0:
                    emit_qk(3)
                elif h == 1:
                    emit_qk(7)
                    if SCORES_FP8:
                        emit_regroup(6)
                        emit_regroup(7)
                elif h >= 4:
                    emit_v(8 + 2 * (h - 4))
                    emit_v(9 + 2 * (h - 4))
            else:
                # chunk-0 c_proj tiles as PE filler for chunk-1 attention
                emit_cproj(h)
                if h == 3:
                    emit_cproj_tail(0)
                elif h == 7:
                    emit_cproj_tail(1, last=True)


def build_nc():
    _install_drain_fix()
    from contextlib import ExitStack

    nc = bacc.Bacc()
    with ExitStack() as ctx:
        tc = ctx.enter_context(tile.TileContext(nc))
        _emit(nc, tc, ctx)
    nc.finalize()  # Bacc: alloc_regs + insert_library_loads happen here
    return nc


def _split_pack(a, scale, ng):
    """Split f32 array [rows, cols] into e4m3 (hi, lo) of scale*a, packed
    [128, ng, 2, cols] so DR group g, slot i, partition p holds row
    256*g + 128*i + p."""
    a = np.asarray(a, dtype=np.float32) * scale
    hi = a.astype(E4_NP)
    lo = (a - hi.astype(np.float32)).astype(E4_NP)
    cols = a.shape[1]

    def pack(x):
        return np.ascontiguousarray(
            x.reshape(ng, 2, 128, cols).transpose(2, 0, 1, 3)
        )

    return pack(hi), pack(lo)


def make_in_maps(inputs, w_attn, b_attn, w_proj, b_proj):
    """Build the 8 per-core input dicts from the full tensors."""
    x = np.asarray(inputs, dtype=np.float32)
    w_attn = np.asarray(w_attn, dtype=np.float32)
    b_attn = np.asarray(b_attn, dtype=np.float32)
    w_proj = np.asarray(w_proj, dtype=np.float32)

    # X^T splits are per batch (shared by the core pair)
    xsp = [_split_pack(x[b].T, 1.0, NG) for b in range(4)]

    in_maps = []
    for c in range(8):
        b, half = c // 2, c % 2
        h0 = half * 8
        cols = np.arange(h0 * 64, h0 * 64 + 512)
        wqk = np.concatenate([w_attn[:, cols], w_attn[:, 1024 + cols]], axis=1)
        wqh, wql = _split_pack(wqk, WS, NG)
        bqk = np.ascontiguousarray(
            (WS * np.concatenate([b_attn[cols], b_attn[1024 + cols]]))
            .reshape(8, 128)
            .T
        )
        vbase = 2048 + h0 * 64
        wvh, wvl = _split_pack(w_attn[:, vbase : vbase + 512], WS, NG)
        bva = (WS * b_attn[vbase : vbase + 512]).reshape(1, 512)
        wph, wpl = _split_pack(w_proj[h0 * 64 : h0 * 64 + 512, :], WS, 2)
        in_maps.append(
            {
                "xh": xsp[b][0],
                "xl": xsp[b][1],
                "wqh": wqh,
                "wql": wql,
                "wvh": wvh,
                "wvl": wvl,
                "wph": wph,
                "wpl": wpl,
                "bqk": np.ascontiguousarray(bqk.astype(np.float32)),
                "bva": np.ascontiguousarray(bva.astype(BF16_NP)),
            }
        )
    return in_maps


_CACHE = {}


def kernel(**inputs):
    nc = _CACHE.get("nc")
    if nc is None:
        nc = _CACHE["nc"] = build_nc()
    in_maps = make_in_maps(
        inputs["inputs"],
        inputs["w_attn"],
        inputs["b_attn"],
        inputs["w_proj"],
        inputs["b_proj"],
    )
    res = run_bass_kernel_spmd(nc, in_maps, core_ids=list(range(8)))
    return gather(res.results, inputs["b_proj"])


def gather(results, b_proj):
    # device output carries the (32*A)·(32*Wp) = 1024x weight scale
    out = np.zeros((4, S, E), dtype=np.float32)
    for b in range(4):
        for c in (2 * b, 2 * b + 1):
            r = results[c]
            # rows 0:1024 come from "out"; the device writes rows 1024:2048
            # only via the per-ct-pair partials in "out2"
            out[b, 0:1024] += r["out"][0:1024].astype(np.float32)
            out[b, 1024:2048] += r["out2"].astype(np.float32).sum(axis=0)
    out *= 1.0 / (WS * WS)
    out += np.asarray(b_proj, dtype=np.float32)[None, None, :]
    return out


# revision 72
# speedup vs baseline: 1.3482x; 1.0488x over previous
"""GPT-2 style causal attention block (B=4, S=2048, E=1024, H=16, D=64) on
8 TRN2 NeuronCores.

Sharding: batch(4) x head-half(2) -> 8 cores, zero on-device communication.
Core c handles batch b=c//2 and heads h0=(c%2)*8 .. h0+7. Each core computes
its qkv column block, attention for its 8 heads, and a partial c_proj
(its 512 rows of w_proj). The partial outputs per batch are summed on the
host during unshard (which also applies the 1/1024 weight-scale and b_proj).

fp8 DoubleRow usage (error-free hi/lo residual splits unless noted):
- qkv: X^T and weights pre-split on the host into e4m3 (hi, lo) packed
  [128, 4, 2, cols]; each DR matmul contracts 256 embedding rows at 0.5
  cyc/col; hh+hl+lh gives 6 column-passes vs bf16's 8. Weights are 32x
  pre-scaled (e4m3 normal range), so Q^T/K^T/V are carried 32-scaled.
- scores: Q^T/K^T quantized to e4m3 (plain, ~1.1e-2 added rel err) and
  DMA-regrouped so head-dim contracts as a DR [32, 2] pack at 0.5 cyc/col.
- c_proj: A^T carried as a 32-scaled e4m3 (hi, lo) pair, w_proj 32x
  pre-scaled and split on the host; ct-pairs contract 256 rows per DR
  matmul (3 split terms = 6 passes vs bf16's 8). The 1/1024 descale and
  b_proj land in the host-side gather.

Attention (per head, q-chunks of 1024): scores^T[k, q] via W-stationary
matmuls, exp on ACT (the dominant ACT cost, ~135us: it bounds how much
the other engines may carry), causality by computing only k<=q 128-tiles
plus a gpsimd affine_select on each diagonal 128-block. attn@V is
REORIENTED: out[q, d] per (q-tile, kt) with P as the stationary operand
pays 64 columns instead of 128 -> half the PE cost of the [d, q] form.
The 8 q-tile accumulators of a chunk pack into ONE PSUM bank [128, 8, 64];
softmax denominators accumulate via rank-1 ones-column matmuls into a
second bank, giving one batched reciprocal per (head, chunk) and a
normalization that is folded into the PSUM-drain copy (per-partition
scalar). A^T is then rebuilt per head-pair by PE transposes (identity
matmul) with psum drains split across DVE/Pool, writing the e4m3 hi/lo
pair that c_proj consumes.

Scheduling: attention bodies priority-boosted over filler (qkv pairs 2-3,
V tiles 8-15, c_proj tiles) which is interleaved into the exp-bound
stretches; during the DMA-paced ramp the qkv groups borrow the idle
attention PSUM banks. PSUM accumulates f32; copies avoid ACT entirely
(exp saturates it) and alternate DVE/Pool.
"""

import re

import ml_dtypes
import numpy as np

import concourse.mybir as mybir
import concourse.tile as tile
from concourse import bacc
from concourse.bass_utils import run_bass_kernel_spmd
from concourse.vector_clock import ScopedClock

F32 = mybir.dt.float32
BF16 = mybir.dt.bfloat16
F8 = mybir.dt.float8e4
BF16_NP = ml_dtypes.bfloat16
E4_NP = ml_dtypes.float8_e4m3
AF = mybir.ActivationFunctionType
DR = mybir.MatmulPerfMode.DoubleRow

S = 2048          # sequence length (per batch)
E = 1024          # embedding dim
HL = 8            # heads per core
D = 64            # head dim
TT = S // 128     # 16 token tiles
NG = 4            # DoubleRow contraction groups of 256 over E
NCH = S // 1024   # 2 q-chunks of 1024
WS = 32.0          # weight pre-scale: q/k/v (and A^T, w_proj) carried 32x
EXP_SCALE = 0.125 / (WS * WS)
PRIO_OFFSET = 800  # attention body scheduled ahead of filler work
SCORES_FP8 = True


def _install_drain_fix():
    """walrus in this container rejects the Tile kernel-tail Drain when it
    carries all semaphore waits on one instruction ("Too many sync wait
    commands"). Emit one wait_ge per semaphore, then a bare drain."""
    if getattr(tile.TileContext, "_drain_fix_installed", False):
        return

    def _split_drain_and_barrier(self, tick_clock, wait_clock):
        nc = self.nc
        probe = mybir.InstDrain(
            name="probe-drain", engine=mybir.EngineType.SP, ins=[], outs=[]
        )
        wait_clock.add_sem_waits(probe, ScopedClock({None: tick_clock.global_clock}))
        waits = re.findall(r"wait:S\[([A-Za-z0-9_]+)\]>=(\d+)", probe.concise())
        handles = {h.name: h for h in self.sems.allocated().values()}
        for name, val in waits:
            nc.sync.wait_ge(handles[name], int(val))
        nc.sync.drain()
        nc.all_engine_barrier()
        popped = nc._tile_sem_poison_stack.pop()
        assert popped is self._sem_poison
        nc.clear_and_free_semaphores(list(self.sems.allocated().values()))
        nc.all_engine_barrier()

    tile.TileContext._drain_and_barrier = _split_drain_and_barrier
    tile.TileContext._drain_fix_installed = True


def _emit(nc, tc, ctx):
    xh_d = nc.declare_dram_parameter("xh", [128, NG, 2, S], F8, isOutput=False)
    xl_d = nc.declare_dram_parameter("xl", [128, NG, 2, S], F8, isOutput=False)
    wqh_d = nc.declare_dram_parameter("wqh", [128, NG, 2, 1024], F8, isOutput=False)
    wql_d = nc.declare_dram_parameter("wql", [128, NG, 2, 1024], F8, isOutput=False)
    wvh_d = nc.declare_dram_parameter("wvh", [128, NG, 2, 512], F8, isOutput=False)
    wvl_d = nc.declare_dram_parameter("wvl", [128, NG, 2, 512], F8, isOutput=False)
    wph_d = nc.declare_dram_parameter("wph", [128, 2, 2, E], F8, isOutput=False)
    wpl_d = nc.declare_dram_parameter("wpl", [128, 2, 2, E], F8, isOutput=False)
    bqk_d = nc.declare_dram_parameter("bqk", [128, 8], F32, isOutput=False)
    bva_d = nc.declare_dram_parameter("bva", [1, 512], BF16, isOutput=False)
    out_d = nc.declare_dram_parameter("out", [S, E], BF16, isOutput=True)
    # tail-region (rows 1024:2048) c_proj partials, one per ct-PAIR;
    # summed on the host together with the core-pair reduction
    out2_d = nc.declare_dram_parameter("out2", [2, 1024, E], BF16, isOutput=True)

    consts = ctx.enter_context(tc.tile_pool(name="consts", bufs=1))
    statics = ctx.enter_context(tc.tile_pool(name="statics", bufs=1))
    ptp = ctx.enter_context(tc.tile_pool(name="ptp", bufs=12))
    rp = ctx.enter_context(tc.tile_pool(name="rp", bufs=4))
    yp = ctx.enter_context(tc.tile_pool(name="yp", bufs=3))
    # PSUM budget (8 banks):
    #   psS 2x[128,1024] = 4 (score tiles: depth 2 so scores(kt+1) overlaps
    #                         exp(kt) — the ACT exp stream must never starve)
    #   paP 2x[128,8,64] = 2 (reoriented attn@V accumulators, 1 bank each)
    #   dnP 1x[128,8]    = 1 (softmax denominators)
    #   psQ 1x[128,512]  = 1 (qkv / c_proj groups + A^T transpose staging)
    psS = ctx.enter_context(tc.tile_pool(name="psS", bufs=2, space="PSUM"))
    paP = ctx.enter_context(tc.tile_pool(name="paP", bufs=2, space="PSUM"))
    dnP = ctx.enter_context(tc.tile_pool(name="dnP", bufs=1, space="PSUM"))
    psQ = ctx.enter_context(tc.tile_pool(name="psQ", bufs=1, space="PSUM"))

    # ---- front section: DMA order matters (the DMA engines are a single
    # serialized resource). The hh-term operands first so qkv starts early,
    # then the lo tensors, wva, and wp (needed last) at the end ----
    xh_sb = statics.tile([128, NG, 2, S], F8)
    xl_sb = statics.tile([128, NG, 2, S], F8)
    wqh_sb = statics.tile([128, NG, 2, 1024], F8)
    wql_sb = statics.tile([128, NG, 2, 1024], F8)
    wvh_sb = statics.tile([128, NG, 2, 512], F8)
    wvl_sb = statics.tile([128, NG, 2, 512], F8)
    wph_sb = statics.tile([128, 2, 2, E], F8)
    wpl_sb = statics.tile([128, 2, 2, E], F8)

    for g in range(NG):
        nc.gpsimd.dma_start(out=wqh_sb[:, g, :, :], in_=wqh_d[:, g, :, :])
        if g == 0:
            nc.sync.dma_start(out=xh_sb[:, 0, 0, :], in_=xh_d[:, 0, 0, :])
            nc.sync.dma_start(out=xh_sb[:, 0, 1, :], in_=xh_d[:, 0, 1, :])
        else:
            nc.sync.dma_start(out=xh_sb[:, g, :, :], in_=xh_d[:, g, :, :])
    for g in range(NG):
        nc.gpsimd.dma_start(out=wql_sb[:, g, :, :], in_=wql_d[:, g, :, :])
    # xl rides both queues so the lh split-terms unlock ~3us earlier
    for g in range(NG):
        (nc.sync if g < 2 else nc.gpsimd).dma_start(
            out=xl_sb[:, g, :, :], in_=xl_d[:, g, :, :]
        )
    for g in range(NG):
        nc.gpsimd.dma_start(out=wvh_sb[:, g, :, :], in_=wvh_d[:, g, :, :])
        nc.gpsimd.dma_start(out=wvl_sb[:, g, :, :], in_=wvl_d[:, g, :, :])
    for g in range(2):
        nc.gpsimd.dma_start(out=wph_sb[:, g, :, :], in_=wph_d[:, g, :, :])
        nc.gpsimd.dma_start(out=wpl_sb[:, g, :, :], in_=wpl_d[:, g, :, :])

    bqk_sb = consts.tile([128, 8], F32)
    nc.sync.dma_start(out=bqk_sb[:, :], in_=bqk_d[:, :])
    bva_sb = consts.tile([1, 512], BF16)
    nc.sync.dma_start(out=bva_sb, in_=bva_d[:])
    ones_sb = consts.tile([1, 128], BF16)
    nc.gpsimd.memset(ones_sb[:], 1.0)
    # denominator column: value 1.0 makes pa*(1/den) carry 32*A (the V 32x
    # scale survives), which is exactly the e4m3 range A^T wants
    dcol_sb = consts.tile([128, 1], BF16)
    nc.gpsimd.memset(dcol_sb[:], 1.0)
    # identity for PE transposes
    ident_sb = consts.tile([128, 128], BF16)
    nc.gpsimd.memset(ident_sb[:], 1.0)
    nc.gpsimd.affine_select(
        out=ident_sb[:],
        in_=ident_sb[:],
        compare_op=mybir.AluOpType.is_equal,
        fill=0.0,
        base=0,
        pattern=[[1, 128]],
        channel_multiplier=-1,
    )

    # ---- qkv Q^T,K^T (W stationary, fp8 DR hi/lo x hi/lo, 32-scaled) paired
    # so head h's Q and K m-tiles arrive together, interleaved with V tiles ----
    qkt_sb = statics.tile([128, 8, S], F8)
    # one [64,...] tile per head pair: head h at partition 32*(h%2),
    # dim1 = Q/K, dim2 = the DoubleRow d-pair slot
    qk2_sb = [statics.tile([64, 2, 2, S], F8, name=f"qk2_{j}") for j in range(4)]
    va_sb = statics.tile([128, TT, 512], BF16)

    # split-product terms in DMA-arrival order: hh first, then hl, lh
    QK_TERMS = ((wqh_sb, xh_sb), (wql_sb, xh_sb), (wqh_sb, xl_sb))
    V_TERMS = ((wvh_sb, xh_sb), (wvl_sb, xh_sb), (wvh_sb, xl_sb))

    def emit_qk(m, ramp=False):
        # during the DMA-paced ramp the attention PSUM banks are still free:
        # spread the first pair's groups across them so more accumulations
        # are in flight per arriving weight tile
        if ramp == "attn":
            pools = [psQ, paP, paP, dnP]
        elif ramp:
            pools = [psS, psS, psS, psS]
        else:
            pools = [psQ] * 4
        tags = {id(psQ): "qk", id(paP): "o", id(dnP): "d", id(psS): "sc"}
        for tch in range(4):
            pqk = pools[tch].tile([128, 512], F32, tag=tags[id(pools[tch])])
            n = 0
            for wsb, xsb in QK_TERMS:
                for g in range(NG):
                    for c in range(2):
                        nc.tensor.matmul(
                            pqk[:, c * 256 : (c + 1) * 256],
                            wsb[:, g, :, m * 128 : (m + 1) * 128],
                            xsb[:, g, :, tch * 512 + c * 256 : tch * 512 + (c + 1) * 256],
                            start=(n == 0),
                            stop=(n == 23),
                            perf_mode=DR,
                        )
                        n += 1
            nc.vector.tensor_scalar_add(
                qkt_sb[:, m, tch * 512 : (tch + 1) * 512], pqk, bqk_sb[:, m : m + 1]
            )

    def emit_regroup(h):
        # partition-regrouping DMAs for the [32,2] d-packed score operands
        po, pr = (h % 2) * 64, 32 * (h % 2)
        t = qk2_sb[h // 2]
        for m, g in ((h // 2, 0), (4 + h // 2, 1)):
            for i in (0, 1):
                nc.sync.dma_start(
                    out=t[pr : pr + 32, g, i, :],
                    in_=qkt_sb[po + 32 * i : po + 32 * i + 32, m, :],
                )

    def emit_v(i, pool=None):
        # never the psS pool: its buf rotation would gate the first score
        # tiles behind the (wv-DMA-gated) V fills
        pool = pool or psQ
        tag = {id(psQ): "qk", id(paP): "o", id(dnP): "d", id(psS): "sc"}[id(pool)]
        pv1 = pool.tile([128, 512], F32, tag=tag)
        n = 0
        for wsb, xsb in V_TERMS:
            for g in range(NG):
                for c in range(2):
                    nc.tensor.matmul(
                        pv1[:, c * 256 : (c + 1) * 256],
                        xsb[:, g, :, i * 128 : (i + 1) * 128],
                        wsb[:, g, :, c * 256 : (c + 1) * 256],
                        start=(n == 0),
                        stop=False,
                        perf_mode=DR,
                    )
                    n += 1
        # V bias as a rank-1 ones-row matmul into the same PSUM group
        for c in range(2):
            nc.tensor.matmul(
                pv1[:, c * 256 : (c + 1) * 256],
                ones_sb[0:1, 0:128],
                bva_sb[0:1, c * 256 : (c + 1) * 256],
                start=False,
                stop=(c == 1),
            )
        nc.vector.tensor_copy(va_sb[:, i, :], pv1)

    # Q/K pairs 0-2 and V tiles 0-7 up front (the hh/hl split terms fill the
    # early-DMA window); pair 3 is deferred into the chunk-0 head loop as PE
    # filler. V0/V1 borrow the score banks (drained before the first score
    # tile's turn in the rotation); V2-7 stream through psQ and may lag —
    # attn@V catches up behind the exp stream.
    emit_qk(0, ramp="attn")    # pair 0 borrows the attention banks
    emit_qk(4, ramp="attn")
    if SCORES_FP8:
        emit_regroup(0)
        emit_regroup(1)
    emit_qk(1, ramp=True)      # pair 1 borrows the score banks
    emit_qk(5, ramp=True)
    if SCORES_FP8:
        emit_regroup(2)
        emit_regroup(3)
    for i in range(TT):
        emit_v(i)

    # ---- attention (q-chunks of 1024), interleaved with c_proj halves ----
    # A^T (32-scaled): bf16 master + e4m3 hi/lo pair for the DR c_proj
    at32_sb = statics.tile([128, 4, S], BF16)
    ath_sb = statics.tile([128, 4, S], F8)
    atl_sb = statics.tile([128, 4, S], F8)
    # [q, c]-oriented normalized attn out per head pair (double-buffered)
    asb_sb = statics.tile([128, 2, NCH, 8, 128], BF16)

    CP_TERMS = ((ath_sb, wph_sb), (ath_sb, wpl_sb), (atl_sb, wph_sb))

    def emit_cproj(i):
        ysb = yp.tile([128, E], BF16, tag="y")
        for ech in range(2):
            py = psQ.tile([128, 512], F32, tag="qk")
            n = 0
            for asb, wsb in CP_TERMS:
                for g in range(2):
                    for c in range(2):
                        nc.tensor.matmul(
                            py[:, c * 256 : (c + 1) * 256],
                            asb[:, 2 * g : 2 * g + 2, i * 128 : (i + 1) * 128],
                            wsb[:, g, :, ech * 512 + c * 256 : ech * 512 + (c + 1) * 256],
                            start=(n == 0),
                            stop=(n == 11),
                            perf_mode=DR,
                        )
                        n += 1
            nc.vector.tensor_copy(ysb[:, ech * 512 : (ech + 1) * 512], py)
            nc.sync.dma_start(
                out=out_d[i * 128 : (i + 1) * 128, ech * 512 : (ech + 1) * 512],
                in_=ysb[:, ech * 512 : (ech + 1) * 512],
            )

    def emit_cproj_tail(ctp, last=False):
        # tail-region (q rows 1024:2048) partial for ct-pair ctp: contraction
        # is one DR group (256 rows), host sums the 2 partials. The final
        # tail is a pure epilogue: attention banks (paP) are free by then and
        # ACT is exp-idle, so spread psum groups and drains across both.
        for i in range(8, 16):
            y2 = yp.tile([128, E], BF16, tag="y")
            for ech in range(2):
                if last:
                    py = paP.tile([128, 512], F32, tag="o")
                else:
                    py = psQ.tile([128, 512], F32, tag="qk")
                n = 0
                for asb, wsb in CP_TERMS:
                    for c in range(2):
                        nc.tensor.matmul(
                            py[:, c * 256 : (c + 1) * 256],
                            asb[:, 2 * ctp : 2 * ctp + 2, i * 128 : (i + 1) * 128],
                            wsb[:, ctp, :, ech * 512 + c * 256 : ech * 512 + (c + 1) * 256],
                            start=(n == 0),
                            stop=(n == 5),
                            perf_mode=DR,
                        )
                        n += 1
                if last and (i + ech) % 2 == 0:
                    nc.scalar.copy(out=y2[:, ech * 512 : (ech + 1) * 512], in_=py)
                else:
                    nc.vector.tensor_copy(y2[:, ech * 512 : (ech + 1) * 512], py)
            nc.sync.dma_start(
                out=out2_d[ctp, (i - 8) * 128 : (i - 7) * 128, :], in_=y2
            )

    def emit_transposes(hp, j):
        # rebuild A^T for a completed (head pair, chunk): PE transpose per
        # q-tile, DVE drains the bf16 master, Pool (SBUF-only) derives the
        # e4m3 hi/lo pair c_proj consumes
        ctx_t = tc.high_priority(offset=PRIO_OFFSET)
        ctx_t.__enter__()
        for qc in range(8):
            qs = j * 1024 + qc * 128
            pst = (dnP if j == 0 else psQ).tile(
                [128, 128], BF16, tag="d" if j == 0 else "qk"
            )
            nc.tensor.transpose(pst, asb_sb[:, hp % 2, j, qc, :], ident_sb)
            nc.vector.tensor_copy(at32_sb[:, hp, qs : qs + 128], pst)
            nc.gpsimd.tensor_copy(
                ath_sb[:, hp, qs : qs + 128], at32_sb[:, hp, qs : qs + 128]
            )
            nc.gpsimd.tensor_sub(
                atl_sb[:, hp, qs : qs + 128],
                at32_sb[:, hp, qs : qs + 128],
                ath_sb[:, hp, qs : qs + 128],
            )
        ctx_t.__exit__(None, None, None)

    # head-major, chunks inner: chunk-1's long exp streams overlap the
    # qkv-heavy prefix so ACT (the bottleneck engine) never starves
    for h in range(HL):
        for j in range(NCH):
            q0 = j * 1024
            nkt = 8 * (j + 1)
            po = (h % 2) * 64
            qm, km = h // 2, 4 + h // 2
            ctx_hp = tc.high_priority(offset=PRIO_OFFSET)
            ctx_hp.__enter__()
            # one bank holds all 8 q-tile accumulators [128, qc, 64];
            # denominators accumulate per q-tile column in dnP
            pa = paP.tile([128, 8, 64], F32, tag="o")
            den = dnP.tile([128, 8], F32, tag="d")
            for kt in range(nkt):
                p = kt - 8 * j
                off = max(0, p * 128)
                qc0 = off // 128
                ps2 = psS.tile([128, 1024], F32, tag="sc")
                if SCORES_FP8:
                    t, pr = qk2_sb[h // 2], 32 * (h % 2)
                    aa = off
                    while aa < 1024:
                        bb = min(1024, (aa // 256 + 1) * 256)
                        nc.tensor.matmul(
                            ps2[:, aa:bb],
                            t[pr : pr + 32, 1, :, kt * 128 : (kt + 1) * 128],
                            t[pr : pr + 32, 0, :, q0 + aa : q0 + bb],
                            start=True,
                            stop=True,
                            perf_mode=DR,
                        )
                        aa = bb
                else:
                    for a, b in ([(off, 512), (512, 1024)] if off < 512 else [(off, 1024)]):
                        nc.tensor.matmul(
                            ps2[:, a:b],
                            qkt_sb[po : po + 64, km, kt * 128 : (kt + 1) * 128],
                            qkt_sb[po : po + 64, qm, q0 + a : q0 + b],
                            start=True,
                            stop=True,
                        )
                pt = ptp.tile([128, 1024], BF16, tag="pt")
                nc.scalar.activation(
                    out=pt[:, off:1024], in_=ps2[:, off:1024], func=AF.Exp,
                    scale=EXP_SCALE,
                )
                if p >= 0:
                    # causal triangle on the diagonal 128-block: keep where
                    # q >= k, zero elsewhere (Pool engine; ACT is exp-bound)
                    nc.gpsimd.affine_select(
                        out=pt[:, off : off + 128],
                        in_=pt[:, off : off + 128],
                        compare_op=mybir.AluOpType.is_ge,
                        fill=0.0,
                        base=0,
                        pattern=[[1, 128]],
                        channel_multiplier=-1,
                    )
                # reoriented attn@V: out[q, d] — P stationary, V moving
                # (64 cols), one matmul per live q-tile; denominator via a
                # rank-1 ones column into the shared den bank
                for qc in range(qc0, 8):
                    qs = qc * 128
                    nc.tensor.matmul(
                        pa[:, qc, :],
                        pt[:, qs : qs + 128],
                        va_sb[:, kt, h * 64 : (h + 1) * 64],
                        start=(kt == 0 and qc == 0),
                        stop=(kt == 8 * j + qc),
                        skip_group_check=True,
                    )
                    nc.tensor.matmul(
                        den[:, qc : qc + 1],
                        pt[:, qs : qs + 128],
                        dcol_sb[:, :],
                        start=(kt == 0 and qc == 0),
                        stop=(kt == 8 * j + qc),
                        skip_group_check=True,
                    )
            rinv = rp.tile([128, 8], F32, tag="ri")
            nc.vector.reciprocal(out=rinv, in_=den)
            # normalization folded into the PSUM drain: asb[q, c] = pa * rinv
            # (per-partition scalar). GPSIMD can't touch PSUM, so DVE only.
            for qc in range(8):
                nc.vector.tensor_scalar_mul(
                    asb_sb[:, (h // 2) % 2, j, qc, po : po + 64],
                    pa[:, qc, :],
                    rinv[:, qc : qc + 1],
                )
            ctx_hp.__exit__(None, None, None)
            if h == 7 and j == 0:
                # full chunk-0 A^T is complete: c_proj rows 0:1024 can run
                # as PE filler under head 7's chunk-1 exp stream. The hp3
                # transposes must come first.
                emit_transposes(3, 0)
                for i in range(8):
                    emit_cproj(i)
        if h % 2 == 1:
            hp = h // 2
            if not (h == 7):
                emit_transposes(hp, 0)
            emit_transposes(hp, 1)
        # deferred qkv as PE filler (pair 3 for heads 6-7); the c_proj
        # tail halves run as soon as their A^T column blocks complete
        if h == 0:
            emit_qk(2)
            emit_qk(6)
            if SCORES_FP8:
                emit_regroup(4)
                emit_regroup(5)
        elif h == 1:
            emit_qk(3)
            emit_qk(7)
            if SCORES_FP8:
                emit_regroup(6)
                emit_regroup(7)
        elif h == 3:
            emit_cproj_tail(0)
    emit_cproj_tail(1, last=True)


def build_nc():
    _install_drain_fix()
    from contextlib import ExitStack

    nc = bacc.Bacc()
    with ExitStack() as ctx:
        tc = ctx.enter_context(tile.TileContext(nc))
        _emit(nc, tc, ctx)
    nc.finalize()  # Bacc: alloc_regs + insert_library_loads happen here
    return nc


def _split_pack(a, scale, ng):
    """Split f32 array [rows, cols] into e4m3 (hi, lo) of scale*a, packed
    [128, ng, 2, cols] so DR group g, slot i, partition p holds row
    256*g + 128*i + p."""
    a = np.asarray(a, dtype=np.float32) * scale
    hi = a.astype(E4_NP)
    lo = (a - hi.astype(np.float32)).astype(E4_NP)
    cols = a.shape[1]

    def pack(x):
        return np.ascontiguousarray(
            x.reshape(ng, 2, 128, cols).transpose(2, 0, 1, 3)
        )

    return pack(hi), pack(lo)


def make_in_maps(inputs, w_attn, b_attn, w_proj, b_proj):
    """Build the 8 per-core input dicts from the full tensors."""
    x = np.asarray(inputs, dtype=np.float32)
    w_attn = np.asarray(w_attn, dtype=np.float32)
    b_attn = np.asarray(b_attn, dtype=np.float32)
    w_proj = np.asarray(w_proj, dtype=np.float32)

    # X^T splits are per batch (shared by the core pair)
    xsp = [_split_pack(x[b].T, 1.0, NG) for b in range(4)]

    in_maps = []
    for c in range(8):
        b, half = c // 2, c % 2
        h0 = half * 8
        cols = np.arange(h0 * 64, h0 * 64 + 512)
        wqk = np.concatenate([w_attn[:, cols], w_attn[:, 1024 + cols]], axis=1)
        wqh, wql = _split_pack(wqk, WS, NG)
        bqk = np.ascontiguousarray(
            (WS * np.concatenate([b_attn[cols], b_attn[1024 + cols]]))
            .reshape(8, 128)
            .T
        )
        vbase = 2048 + h0 * 64
        wvh, wvl = _split_pack(w_attn[:, vbase : vbase + 512], WS, NG)
        bva = (WS * b_attn[vbase : vbase + 512]).reshape(1, 512)
        wph, wpl = _split_pack(w_proj[h0 * 64 : h0 * 64 + 512, :], WS, 2)
        in_maps.append(
            {
                "xh": xsp[b][0],
                "xl": xsp[b][1],
                "wqh": wqh,
                "wql": wql,
                "wvh": wvh,
                "wvl": wvl,
                "wph": wph,
                "wpl": wpl,
                "bqk": np.ascontiguousarray(bqk.astype(np.float32)),
                "bva": np.ascontiguousarray(bva.astype(BF16_NP)),
            }
        )
    return in_maps


_CACHE = {}


def kernel(**inputs):
    nc = _CACHE.get("nc")
    if nc is None:
        nc = _CACHE["nc"] = build_nc()
    in_maps = make_in_maps(
        inputs["inputs"],
        inputs["w_attn"],
        inputs["b_attn"],
        inputs["w_proj"],
        inputs["b_proj"],
    )
    res = run_bass_kernel_spmd(nc, in_maps, core_ids=list(range(8)))
    return gather(res.results, inputs["b_proj"])


def gather(results, b_proj):
    # device output carries the (32*A)·(32*Wp) = 1024x weight scale
    out = np.zeros((4, S, E), dtype=np.float32)
    for b in range(4):
        for c in (2 * b, 2 * b + 1):
            r = results[c]
            # rows 0:1024 come from "out"; the device writes rows 1024:2048
            # only via the per-ct-pair partials in "out2"
            out[b, 0:1024] += r["out"][0:1024].astype(np.float32)
            out[b, 1024:2048] += r["out2"].astype(np.float32).sum(axis=0)
    out *= 1.0 / (WS * WS)
    out += np.asarray(b_proj, dtype=np.float32)[None, None, :]
    return out


# revision 89
# speedup vs baseline: 1.4179x; 1.0518x over previous
"""GPT-2 style causal attention block (B=4, S=2048, E=1024, H=16, D=64) on
8 TRN2 NeuronCores.

Sharding: batch(4) x head-half(2) -> 8 cores, zero on-device communication.
Core c handles batch b=c//2 and heads h0=(c%2)*8 .. h0+7. Each core computes
its qkv column block, attention for its 8 heads, and a partial c_proj
(its 512 rows of w_proj). The partial outputs per batch are summed on the
host during unshard (which also applies the 1/1024 weight-scale and b_proj).

fp8 DoubleRow usage (error-free hi/lo residual splits unless noted):
- qkv: X^T and weights pre-split on the host into e4m3 (hi, lo) packed
  [128, 4, 2, cols]; each DR matmul contracts 256 embedding rows at 0.5
  cyc/col; hh+hl+lh gives 6 column-passes vs bf16's 8. Weights are 32x
  pre-scaled (e4m3 normal range), so Q^T/K^T/V are carried 32-scaled.
- scores: Q^T/K^T quantized to e4m3 (plain, ~1.1e-2 added rel err) and
  DMA-regrouped so head-dim contracts as a DR [32, 2] pack at 0.5 cyc/col.
- c_proj: A^T carried as a 32-scaled e4m3 (hi, lo) pair, w_proj 32x
  pre-scaled and split on the host; ct-pairs contract 256 rows per DR
  matmul (3 split terms = 6 passes vs bf16's 8). The 1/1024 descale and
  b_proj land in the host-side gather.

Attention (per head, q-chunks of 1024): scores^T[k, q] via W-stationary
matmuls, exp on ACT (the dominant ACT cost, ~135us: it bounds how much
the other engines may carry), causality by computing only k<=q 128-tiles
plus a gpsimd affine_select on each diagonal 128-block. attn@V is
REORIENTED: out[q, d] per (q-tile, kt) with P as the stationary operand
pays 64 columns instead of 128 -> half the PE cost of the [d, q] form.
The 8 q-tile accumulators of a chunk pack into ONE PSUM bank [128, 8, 64];
softmax denominators accumulate via rank-1 ones-column matmuls into a
second bank, giving one batched reciprocal per (head, chunk) and a
normalization that is folded into the PSUM-drain copy (per-partition
scalar). A^T is then rebuilt per head-pair by PE transposes (identity
matmul) with psum drains split across DVE/Pool, writing the e4m3 hi/lo
pair that c_proj consumes.

Scheduling: attention bodies priority-boosted over filler (qkv pairs 2-3,
V tiles 8-15, c_proj tiles) which is interleaved into the exp-bound
stretches; during the DMA-paced ramp the qkv groups borrow the idle
attention PSUM banks. PSUM accumulates f32; copies avoid ACT entirely
(exp saturates it) and alternate DVE/Pool.
"""

import re

import ml_dtypes
import numpy as np

import concourse.mybir as mybir
import concourse.tile as tile
from concourse import bacc
from concourse.bass_utils import run_bass_kernel_spmd
from concourse.vector_clock import ScopedClock

F32 = mybir.dt.float32
BF16 = mybir.dt.bfloat16
F8 = mybir.dt.float8e4
BF16_NP = ml_dtypes.bfloat16
E4_NP = ml_dtypes.float8_e4m3
AF = mybir.ActivationFunctionType
DR = mybir.MatmulPerfMode.DoubleRow

S = 2048          # sequence length (per batch)
E = 1024          # embedding dim
HL = 8            # heads per core
D = 64            # head dim
TT = S // 128     # 16 token tiles
NG = 4            # DoubleRow contraction groups of 256 over E
NCH = S // 1024   # 2 q-chunks of 1024
WS = 32.0          # weight pre-scale: q/k/v (and A^T, w_proj) carried 32x
EXP_SCALE = 0.125 / (WS * WS)
PRIO_OFFSET = 800  # attention body scheduled ahead of filler work
SCORES_FP8 = True


def _install_drain_fix():
    """walrus in this container rejects the Tile kernel-tail Drain when it
    carries all semaphore waits on one instruction ("Too many sync wait
    commands"). Emit one wait_ge per semaphore, then a bare drain."""
    if getattr(tile.TileContext, "_drain_fix_installed", False):
        return

    def _split_drain_and_barrier(self, tick_clock, wait_clock):
        nc = self.nc
        probe = mybir.InstDrain(
            name="probe-drain", engine=mybir.EngineType.SP, ins=[], outs=[]
        )
        wait_clock.add_sem_waits(probe, ScopedClock({None: tick_clock.global_clock}))
        waits = re.findall(r"wait:S\[([A-Za-z0-9_]+)\]>=(\d+)", probe.concise())
        handles = {h.name: h for h in self.sems.allocated().values()}
        for name, val in waits:
            nc.sync.wait_ge(handles[name], int(val))
        nc.sync.drain()
        nc.all_engine_barrier()
        popped = nc._tile_sem_poison_stack.pop()
        assert popped is self._sem_poison
        nc.clear_and_free_semaphores(list(self.sems.allocated().values()))
        nc.all_engine_barrier()

    tile.TileContext._drain_and_barrier = _split_drain_and_barrier
    tile.TileContext._drain_fix_installed = True


def _emit(nc, tc, ctx):
    xh_d = nc.declare_dram_parameter("xh", [128, NG, 2, S], F8, isOutput=False)
    xl_d = nc.declare_dram_parameter("xl", [128, NG, 2, S], F8, isOutput=False)
    wqh_d = nc.declare_dram_parameter("wqh", [128, NG, 2, 1024], F8, isOutput=False)
    wql_d = nc.declare_dram_parameter("wql", [128, NG, 2, 1024], F8, isOutput=False)
    wvh_d = nc.declare_dram_parameter("wvh", [128, NG, 2, 512], F8, isOutput=False)
    wvl_d = nc.declare_dram_parameter("wvl", [128, NG, 2, 512], F8, isOutput=False)
    wph_d = nc.declare_dram_parameter("wph", [128, 2, 2, E], F8, isOutput=False)
    wpl_d = nc.declare_dram_parameter("wpl", [128, 2, 2, E], F8, isOutput=False)
    bqk_d = nc.declare_dram_parameter("bqk", [128, 8], F32, isOutput=False)
    bva_d = nc.declare_dram_parameter("bva", [1, 512], BF16, isOutput=False)
    out_d = nc.declare_dram_parameter("out", [S, E], BF16, isOutput=True)
    # tail-region (rows 1024:2048) c_proj partials, one per ct-PAIR;
    # summed on the host together with the core-pair reduction
    out2_d = nc.declare_dram_parameter("out2", [2, 1024, E], BF16, isOutput=True)

    consts = ctx.enter_context(tc.tile_pool(name="consts", bufs=1))
    statics = ctx.enter_context(tc.tile_pool(name="statics", bufs=1))
    ptp = ctx.enter_context(tc.tile_pool(name="ptp", bufs=12))
    rp = ctx.enter_context(tc.tile_pool(name="rp", bufs=4))
    yp = ctx.enter_context(tc.tile_pool(name="yp", bufs=4))
    # PSUM budget (8 banks):
    #   psS 2x[128,1024] = 4 (score tiles: depth 2 so scores(kt+1) overlaps
    #                         exp(kt) — the ACT exp stream must never starve)
    #   paP 2x[128,8,64] = 2 (reoriented attn@V accumulators, 1 bank each)
    #   dnP 1x[128,8]    = 1 (softmax denominators)
    #   psQ 1x[128,512]  = 1 (qkv / c_proj groups + A^T transpose staging)
    psS = ctx.enter_context(tc.tile_pool(name="psS", bufs=2, space="PSUM"))
    paP = ctx.enter_context(tc.tile_pool(name="paP", bufs=2, space="PSUM"))
    dnP = ctx.enter_context(tc.tile_pool(name="dnP", bufs=1, space="PSUM"))
    psQ = ctx.enter_context(tc.tile_pool(name="psQ", bufs=1, space="PSUM"))

    # ---- front section: DMA order matters (the DMA engines are a single
    # serialized resource). The hh-term operands first so qkv starts early,
    # then the lo tensors, wva, and wp (needed last) at the end ----
    xh_sb = statics.tile([128, NG, 2, S], F8)
    xl_sb = statics.tile([128, NG, 2, S], F8)
    wqh_sb = statics.tile([128, NG, 2, 1024], F8)
    wql_sb = statics.tile([128, NG, 2, 1024], F8)
    wvh_sb = statics.tile([128, NG, 2, 512], F8)
    wvl_sb = statics.tile([128, NG, 2, 512], F8)
    wph_sb = statics.tile([128, 2, 2, E], F8)
    wpl_sb = statics.tile([128, 2, 2, E], F8)

    for g in range(NG):
        nc.gpsimd.dma_start(out=wqh_sb[:, g, :, :], in_=wqh_d[:, g, :, :])
        if g == 0:
            nc.sync.dma_start(out=xh_sb[:, 0, 0, :], in_=xh_d[:, 0, 0, :])
            nc.sync.dma_start(out=xh_sb[:, 0, 1, :], in_=xh_d[:, 0, 1, :])
        else:
            nc.sync.dma_start(out=xh_sb[:, g, :, :], in_=xh_d[:, g, :, :])
    for g in range(NG):
        nc.gpsimd.dma_start(out=wql_sb[:, g, :, :], in_=wql_d[:, g, :, :])
    # xl rides both queues so the lh split-terms unlock ~3us earlier
    for g in range(NG):
        (nc.sync if g < 2 else nc.gpsimd).dma_start(
            out=xl_sb[:, g, :, :], in_=xl_d[:, g, :, :]
        )
    for g in range(NG):
        nc.gpsimd.dma_start(out=wvh_sb[:, g, :, :], in_=wvh_d[:, g, :, :])
        nc.gpsimd.dma_start(out=wvl_sb[:, g, :, :], in_=wvl_d[:, g, :, :])
    for g in range(2):
        nc.gpsimd.dma_start(out=wph_sb[:, g, :, :], in_=wph_d[:, g, :, :])
        nc.gpsimd.dma_start(out=wpl_sb[:, g, :, :], in_=wpl_d[:, g, :, :])

    bqk_sb = consts.tile([128, 8], F32)
    nc.sync.dma_start(out=bqk_sb[:, :], in_=bqk_d[:, :])
    bva_sb = consts.tile([1, 512], BF16)
    nc.sync.dma_start(out=bva_sb, in_=bva_d[:])
    ones_sb = consts.tile([1, 512], BF16)
    nc.gpsimd.memset(ones_sb[:], 1.0)
    # denominator column: value 1.0 makes pa*(1/den) carry 32*A (the V 32x
    # scale survives), which is exactly the e4m3 range A^T wants
    dcol_sb = consts.tile([128, 1], BF16)
    nc.gpsimd.memset(dcol_sb[:], 1.0)
    # identity for PE transposes
    ident_sb = consts.tile([128, 128], BF16)
    nc.gpsimd.memset(ident_sb[:], 1.0)
    nc.gpsimd.affine_select(
        out=ident_sb[:],
        in_=ident_sb[:],
        compare_op=mybir.AluOpType.is_equal,
        fill=0.0,
        base=0,
        pattern=[[1, 128]],
        channel_multiplier=-1,
    )

    # ---- qkv Q^T,K^T (W stationary, fp8 DR hi/lo x hi/lo, 32-scaled) paired
    # so head h's Q and K m-tiles arrive together, interleaved with V tiles ----
    qkt_sb = statics.tile([128, 8, S], F8)
    # one [64,...] tile per head pair: head h at partition 32*(h%2),
    # dim1 = Q/K, dim2 = the DoubleRow d-pair slot
    qk2_sb = [statics.tile([64, 2, 2, S], F8, name=f"qk2_{j}") for j in range(4)]
    va_sb = statics.tile([128, TT, 512], BF16)

    # split-product terms in DMA-arrival order: hh first, then hl, lh
    QK_TERMS = ((wqh_sb, xh_sb), (wql_sb, xh_sb), (wqh_sb, xl_sb))
    V_TERMS = ((wvh_sb, xh_sb), (wvl_sb, xh_sb), (wvh_sb, xl_sb))

    def emit_qk(m, ramp=False):
        # during the DMA-paced ramp the attention PSUM banks are still free:
        # spread the first pair's groups across them so more accumulations
        # are in flight per arriving weight tile
        if ramp == "attn":
            pools = [psQ, paP, paP, dnP]
        elif ramp:
            pools = [psS, psS, psS, psS]
        else:
            pools = [psQ] * 4
        tags = {id(psQ): "qk", id(paP): "o", id(dnP): "d", id(psS): "sc"}
        for tch in range(4):
            pqk = pools[tch].tile([128, 512], F32, tag=tags[id(pools[tch])])
            n = 0
            for wsb, xsb in QK_TERMS:
                for g in range(NG):
                    for c in range(2):
                        nc.tensor.matmul(
                            pqk[:, c * 256 : (c + 1) * 256],
                            wsb[:, g, :, m * 128 : (m + 1) * 128],
                            xsb[:, g, :, tch * 512 + c * 256 : tch * 512 + (c + 1) * 256],
                            start=(n == 0),
                            stop=(n == 23),
                            perf_mode=DR,
                        )
                        n += 1
            nc.vector.tensor_scalar_add(
                qkt_sb[:, m, tch * 512 : (tch + 1) * 512], pqk, bqk_sb[:, m : m + 1]
            )

    def emit_regroup(h):
        # partition-regrouping DMAs for the [32,2] d-packed score operands
        po, pr = (h % 2) * 64, 32 * (h % 2)
        t = qk2_sb[h // 2]
        for m, g in ((h // 2, 0), (4 + h // 2, 1)):
            for tch in range(4):
                cs = slice(tch * 512, (tch + 1) * 512)
                for i in (0, 1):
                    # per-512-col pieces: the first score tiles only need the
                    # early columns, so they unblock right after tch0's quant
                    nc.sync.dma_start(
                        out=t[pr : pr + 32, g, i, cs],
                        in_=qkt_sb[po + 32 * i : po + 32 * i + 32, m, cs],
                    )

    def emit_v(i, pool=None):
        # never the psS pool: its buf rotation would gate the first score
        # tiles behind the (wv-DMA-gated) V fills
        pool = pool or psQ
        tag = {id(psQ): "qk", id(paP): "o", id(dnP): "d", id(psS): "sc"}[id(pool)]
        pv1 = pool.tile([128, 512], F32, tag=tag)
        n = 0
        for wsb, xsb in V_TERMS:
            for g in range(NG):
                for c in range(2):
                    nc.tensor.matmul(
                        pv1[:, c * 256 : (c + 1) * 256],
                        xsb[:, g, :, i * 128 : (i + 1) * 128],
                        wsb[:, g, :, c * 256 : (c + 1) * 256],
                        start=(n == 0),
                        stop=False,
                        perf_mode=DR,
                    )
                    n += 1
        # V bias as a rank-1 ones-row matmul into the same PSUM group
        for c in range(2):
            nc.tensor.matmul(
                pv1[:, c * 256 : (c + 1) * 256],
                ones_sb[0:1, 0:128],
                bva_sb[0:1, c * 256 : (c + 1) * 256],
                start=False,
                stop=(c == 1),
            )
        nc.vector.tensor_copy(va_sb[:, i, :], pv1)

    # Q/K pairs 0-2 and V tiles 0-7 up front (the hh/hl split terms fill the
    # early-DMA window); pair 3 is deferred into the chunk-0 head loop as PE
    # filler. V0/V1 borrow the score banks (drained before the first score
    # tile's turn in the rotation); V2-7 stream through psQ and may lag —
    # attn@V catches up behind the exp stream.
    emit_qk(0, ramp="attn")    # pair 0 borrows the attention banks
    emit_qk(4, ramp="attn")
    if SCORES_FP8:
        emit_regroup(0)
        emit_regroup(1)
    emit_qk(1, ramp=True)      # pair 1 borrows the score banks
    emit_qk(5, ramp=True)
    if SCORES_FP8:
        emit_regroup(2)
        emit_regroup(3)
    for i in range(TT):
        emit_v(i)

    # ---- attention (q-chunks of 1024), interleaved with c_proj halves ----
    # A^T (32-scaled): bf16 master + e4m3 hi/lo pair for the DR c_proj
    at32_sb = statics.tile([128, 4, S], BF16)
    ath_sb = statics.tile([128, 4, S], F8)
    atl_sb = statics.tile([128, 4, S], F8)
    # [q, c]-oriented normalized attn out per head pair (double-buffered)
    asb_sb = statics.tile([128, 2, NCH, 8, 128], BF16)

    CP_TERMS = ((ath_sb, wph_sb), (ath_sb, wpl_sb), (atl_sb, wph_sb))

    def emit_cproj(i, last=False):
        ysb = yp.tile([128, E], BF16, tag="y")
        for ech in range(2):
            if last:
                # epilogue: attention banks are free — spread the groups
                pool = (paP, psS)[(2 * i + ech) % 2]
                py = pool.tile([128, 512], F32, tag="o" if pool is paP else "sc")
            else:
                py = psQ.tile([128, 512], F32, tag="qk")
            n = 0
            for asb, wsb in CP_TERMS:
                for g in range(2):
                    for c in range(2):
                        nc.tensor.matmul(
                            py[:, c * 256 : (c + 1) * 256],
                            asb[:, 2 * g : 2 * g + 2, i * 128 : (i + 1) * 128],
                            wsb[:, g, :, ech * 512 + c * 256 : ech * 512 + (c + 1) * 256],
                            start=(n == 0),
                            stop=(n == 11),
                            perf_mode=DR,
                        )
                        n += 1
            if last and ech == 0:
                nc.scalar.copy(out=ysb[:, ech * 512 : (ech + 1) * 512], in_=py)
            else:
                nc.vector.tensor_copy(ysb[:, ech * 512 : (ech + 1) * 512], py)
            nc.sync.dma_start(
                out=out_d[i * 128 : (i + 1) * 128, ech * 512 : (ech + 1) * 512],
                in_=ysb[:, ech * 512 : (ech + 1) * 512],
            )

    def emit_cproj_tail(ctp, last=False):
        # tail-region (q rows 1024:2048) partial for ct-pair ctp: contraction
        # is one DR group (256 rows), host sums the 2 partials. The final
        # tail is a pure epilogue: attention banks (paP) are free by then and
        # ACT is exp-idle, so spread psum groups and drains across both.
        for i in range(8, 16):
            y2 = yp.tile([128, E], BF16, tag="y")
            for ech in range(2):
                if last:
                    pool = (paP, psS)[(2 * i + ech) % 2]
                    py = pool.tile(
                        [128, 512], F32, tag="o" if pool is paP else "sc"
                    )
                else:
                    py = psQ.tile([128, 512], F32, tag="qk")
                n = 0
                for asb, wsb in CP_TERMS:
                    for c in range(2):
                        nc.tensor.matmul(
                            py[:, c * 256 : (c + 1) * 256],
                            asb[:, 2 * ctp : 2 * ctp + 2, i * 128 : (i + 1) * 128],
                            wsb[:, ctp, :, ech * 512 + c * 256 : ech * 512 + (c + 1) * 256],
                            start=(n == 0),
                            stop=(n == 5),
                            perf_mode=DR,
                        )
                        n += 1
                if last and (i + ech) % 2 == 0:
                    nc.scalar.copy(out=y2[:, ech * 512 : (ech + 1) * 512], in_=py)
                else:
                    nc.vector.tensor_copy(y2[:, ech * 512 : (ech + 1) * 512], py)
            nc.sync.dma_start(
                out=out2_d[ctp, (i - 8) * 128 : (i - 7) * 128, :], in_=y2
            )

    def emit_transposes(hp, j):
        # rebuild A^T for a completed (head pair, chunk): PE transpose per
        # q-tile, DVE drains the bf16 master, Pool (SBUF-only) derives the
        # e4m3 hi/lo pair c_proj consumes
        ctx_t = tc.high_priority(offset=PRIO_OFFSET)
        ctx_t.__enter__()
        for qc in range(8):
            qs = j * 1024 + qc * 128
            pst = (dnP if j == 0 else psQ).tile(
                [128, 128], BF16, tag="d" if j == 0 else "qk"
            )
            nc.tensor.transpose(pst, asb_sb[:, hp % 2, j, qc, :], ident_sb)
            nc.vector.tensor_copy(at32_sb[:, hp, qs : qs + 128], pst)
            nc.gpsimd.tensor_copy(
                ath_sb[:, hp, qs : qs + 128], at32_sb[:, hp, qs : qs + 128]
            )
            nc.gpsimd.tensor_sub(
                atl_sb[:, hp, qs : qs + 128],
                at32_sb[:, hp, qs : qs + 128],
                ath_sb[:, hp, qs : qs + 128],
            )
        ctx_t.__exit__(None, None, None)

    # head-major, chunks inner: chunk-1's long exp streams overlap the
    # qkv-heavy prefix so ACT (the bottleneck engine) never starves
    for h in range(HL):
        for j in range(NCH):
            q0 = j * 1024
            nkt = 8 * (j + 1)
            po = (h % 2) * 64
            qm, km = h // 2, 4 + h // 2
            ctx_hp = tc.high_priority(offset=PRIO_OFFSET)
            ctx_hp.__enter__()
            # one bank holds all 8 q-tile accumulators [128, qc, 64];
            # denominators accumulate per q-tile column in dnP
            pa = paP.tile([128, 8, 64], F32, tag="o")
            den = dnP.tile([128, 8], F32, tag="d")
            for kt in range(nkt):
                p = kt - 8 * j
                off = max(0, p * 128)
                qc0 = off // 128
                ps2 = psS.tile([128, 1024], F32, tag="sc")
                if SCORES_FP8:
                    t, pr = qk2_sb[h // 2], 32 * (h % 2)
                    aa = off
                    while aa < 1024:
                        bb = min(1024, (aa // 256 + 1) * 256)
                        nc.tensor.matmul(
                            ps2[:, aa:bb],
                            t[pr : pr + 32, 1, :, kt * 128 : (kt + 1) * 128],
                            t[pr : pr + 32, 0, :, q0 + aa : q0 + bb],
                            start=True,
                            stop=True,
                            perf_mode=DR,
                        )
                        aa = bb
                else:
                    for a, b in ([(off, 512), (512, 1024)] if off < 512 else [(off, 1024)]):
                        nc.tensor.matmul(
                            ps2[:, a:b],
                            qkt_sb[po : po + 64, km, kt * 128 : (kt + 1) * 128],
                            qkt_sb[po : po + 64, qm, q0 + a : q0 + b],
                            start=True,
                            stop=True,
                        )
                pt = ptp.tile([128, 1024], BF16, tag="pt")
                nc.scalar.activation(
                    out=pt[:, off:1024], in_=ps2[:, off:1024], func=AF.Exp,
                    scale=EXP_SCALE,
                )
                if p >= 0:
                    # causal triangle on the diagonal 128-block: keep where
                    # q >= k, zero elsewhere (Pool engine; ACT is exp-bound)
                    nc.gpsimd.affine_select(
                        out=pt[:, off : off + 128],
                        in_=pt[:, off : off + 128],
                        compare_op=mybir.AluOpType.is_ge,
                        fill=0.0,
                        base=0,
                        pattern=[[1, 128]],
                        channel_multiplier=-1,
                    )
                # reoriented attn@V: out[q, d] — P stationary, V moving
                # (64 cols), one matmul per live q-tile; denominator via a
                # rank-1 ones column into the shared den bank
                for qc in range(qc0, 8):
                    qs = qc * 128
                    nc.tensor.matmul(
                        pa[:, qc, :],
                        pt[:, qs : qs + 128],
                        va_sb[:, kt, h * 64 : (h + 1) * 64],
                        start=(kt == 0 and qc == 0),
                        stop=(kt == 8 * j + qc),
                        skip_group_check=True,
                    )
                    nc.tensor.matmul(
                        den[:, qc : qc + 1],
                        pt[:, qs : qs + 128],
                        dcol_sb[:, :],
                        start=(kt == 0 and qc == 0),
                        stop=(kt == 8 * j + qc),
                        skip_group_check=True,
                    )
            rinv = rp.tile([128, 8], F32, tag="ri")
            nc.vector.reciprocal(out=rinv, in_=den)
            # normalization folded into the PSUM drain: asb[q, c] = pa * rinv
            # (per-partition scalar). GPSIMD can't touch PSUM, so DVE only.
            for qc in range(8):
                nc.vector.tensor_scalar_mul(
                    asb_sb[:, (h // 2) % 2, j, qc, po : po + 64],
                    pa[:, qc, :],
                    rinv[:, qc : qc + 1],
                )
            ctx_hp.__exit__(None, None, None)
            if h == 7 and j == 0:
                # full chunk-0 A^T is complete: c_proj rows 0:1024 can run
                # as PE filler under head 7's chunk-1 exp stream. The hp3
                # transposes must come first.
                emit_transposes(3, 0)
                for i in range(4):
                    emit_cproj(i)
        if h % 2 == 1:
            hp = h // 2
            if not (h == 7):
                emit_transposes(hp, 0)
            emit_transposes(hp, 1)
        # deferred qkv as PE filler (pair 3 for heads 6-7); the c_proj
        # tail halves run as soon as their A^T column blocks complete
        if h == 0:
            emit_qk(2)
            emit_qk(6)
            if SCORES_FP8:
                emit_regroup(4)
                emit_regroup(5)
        elif h == 1:
            emit_qk(3)
            emit_qk(7)
            if SCORES_FP8:
                emit_regroup(6)
                emit_regroup(7)
        elif h == 3:
            emit_cproj_tail(0)
    for i in range(4, 8):
        emit_cproj(i, last=True)
    emit_cproj_tail(1, last=True)


def build_nc():
    _install_drain_fix()
    from contextlib import ExitStack

    nc = bacc.Bacc()
    with ExitStack() as ctx:
        tc = ctx.enter_context(tile.TileContext(nc))
        _emit(nc, tc, ctx)
    nc.finalize()  # Bacc: alloc_regs + insert_library_loads happen here
    return nc


def _split_pack(a, scale, ng):
    """Split f32 array [rows, cols] into e4m3 (hi, lo) of scale*a, packed
    [128, ng, 2, cols] so DR group g, slot i, partition p holds row
    256*g + 128*i + p."""
    a = np.asarray(a, dtype=np.float32) * scale
    hi = a.astype(E4_NP)
    lo = (a - hi.astype(np.float32)).astype(E4_NP)
    cols = a.shape[1]

    def pack(x):
        return np.ascontiguousarray(
            x.reshape(ng, 2, 128, cols).transpose(2, 0, 1, 3)
        )

    return pack(hi), pack(lo)


def make_in_maps(inputs, w_attn, b_attn, w_proj, b_proj):
    """Build the 8 per-core input dicts from the full tensors."""
    x = np.asarray(inputs, dtype=np.float32)
    w_attn = np.asarray(w_attn, dtype=np.float32)
    b_attn = np.asarray(b_attn, dtype=np.float32)
    w_proj = np.asarray(w_proj, dtype=np.float32)

    # X^T splits are per batch (shared by the core pair)
    xsp = [_split_pack(x[b].T, 1.0, NG) for b in range(4)]

    in_maps = []
    for c in range(8):
        b, half = c // 2, c % 2
        h0 = half * 8
        cols = np.arange(h0 * 64, h0 * 64 + 512)
        wqk = np.concatenate([w_attn[:, cols], w_attn[:, 1024 + cols]], axis=1)
        wqh, wql = _split_pack(wqk, WS, NG)
        bqk = np.ascontiguousarray(
            (WS * np.concatenate([b_attn[cols], b_attn[1024 + cols]]))
            .reshape(8, 128)
            .T
        )
        vbase = 2048 + h0 * 64
        wvh, wvl = _split_pack(w_attn[:, vbase : vbase + 512], WS, NG)
        bva = (WS * b_attn[vbase : vbase + 512]).reshape(1, 512)
        wph, wpl = _split_pack(w_proj[h0 * 64 : h0 * 64 + 512, :], WS, 2)
        in_maps.append(
            {
                "xh": xsp[b][0],
                "xl": xsp[b][1],
                "wqh": wqh,
                "wql": wql,
                "wvh": wvh,
                "wvl": wvl,
                "wph": wph,
                "wpl": wpl,
                "bqk": np.ascontiguousarray(bqk.astype(np.float32)),
                "bva": np.ascontiguousarray(bva.astype(BF16_NP)),
            }
        )
    return in_maps


_CACHE = {}


def kernel(**inputs):
    nc = _CACHE.get("nc")
    if nc is None:
        nc = _CACHE["nc"] = build_nc()
    in_maps = make_in_maps(
        inputs["inputs"],
        inputs["w_attn"],
        inputs["b_attn"],
        inputs["w_proj"],
        inputs["b_proj"],
    )
    res = run_bass_kernel_spmd(nc, in_maps, core_ids=list(range(8)))
    return gather(res.results, inputs["b_proj"])


def gather(results, b_proj):
    # device output carries the (32*A)·(32*Wp) = 1024x weight scale
    out = np.zeros((4, S, E), dtype=np.float32)
    for b in range(4):
        for c in (2 * b, 2 * b + 1):
            r = results[c]
            # rows 0:1024 come from "out"; the device writes rows 1024:2048
            # only via the per-ct-pair partials in "out2"
            out[b, 0:1024] += r["out"][0:1024].astype(np.float32)
            out[b, 1024:2048] += r["out2"].astype(np.float32).sum(axis=0)
    out *= 1.0 / (WS * WS)
    out += np.asarray(b_proj, dtype=np.float32)[None, None, :]
    return out


# revision 97
# speedup vs baseline: 1.4279x; 1.0070x over previous
"""GPT-2 style causal attention block (B=4, S=2048, E=1024, H=16, D=64) on
8 TRN2 NeuronCores.

Sharding: batch(4) x head-half(2) -> 8 cores, zero on-device communication.
Core c handles batch b=c//2 and heads h0=(c%2)*8 .. h0+7. Each core computes
its qkv column block, attention for its 8 heads, and a partial c_proj
(its 512 rows of w_proj). The partial outputs per batch are summed on the
host during unshard (which also applies the 1/1024 weight-scale and b_proj).

fp8 DoubleRow usage (error-free hi/lo residual splits unless noted):
- qkv: X^T and weights pre-split on the host into e4m3 (hi, lo) packed
  [128, 4, 2, cols]; each DR matmul contracts 256 embedding rows at 0.5
  cyc/col; hh+hl+lh gives 6 column-passes vs bf16's 8. Weights are 32x
  pre-scaled (e4m3 normal range), so Q^T/K^T/V are carried 32-scaled.
- scores: Q^T/K^T quantized to e4m3 (plain, ~1.1e-2 added rel err) and
  DMA-regrouped so head-dim contracts as a DR [32, 2] pack at 0.5 cyc/col.
- c_proj: A^T carried as a 32-scaled e4m3 (hi, lo) pair, w_proj 32x
  pre-scaled and split on the host; ct-pairs contract 256 rows per DR
  matmul (3 split terms = 6 passes vs bf16's 8). The 1/1024 descale and
  b_proj land in the host-side gather.

Attention (per head, q-chunks of 1024): scores^T[k, q] via W-stationary
matmuls, exp on ACT (the dominant ACT cost, ~135us: it bounds how much
the other engines may carry), causality by computing only k<=q 128-tiles
plus a gpsimd affine_select on each diagonal 128-block. attn@V is
REORIENTED: out[q, d] per (q-tile, kt) with P as the stationary operand
pays 64 columns instead of 128 -> half the PE cost of the [d, q] form.
The 8 q-tile accumulators of a chunk pack into ONE PSUM bank [128, 8, 64];
softmax denominators accumulate via rank-1 ones-column matmuls into a
second bank, giving one batched reciprocal per (head, chunk) and a
normalization that is folded into the PSUM-drain copy (per-partition
scalar). A^T is then rebuilt per head-pair by PE transposes (identity
matmul) with psum drains split across DVE/Pool, writing the e4m3 hi/lo
pair that c_proj consumes.

Scheduling: attention bodies priority-boosted over filler (qkv pairs 2-3,
V tiles 8-15, c_proj tiles) which is interleaved into the exp-bound
stretches; during the DMA-paced ramp the qkv groups borrow the idle
attention PSUM banks. PSUM accumulates f32; copies avoid ACT entirely
(exp saturates it) and alternate DVE/Pool.
"""

import re

import ml_dtypes
import numpy as np

import concourse.mybir as mybir
import concourse.tile as tile
from concourse import bacc
from concourse.bass_utils import run_bass_kernel_spmd
from concourse.vector_clock import ScopedClock

F32 = mybir.dt.float32
BF16 = mybir.dt.bfloat16
F8 = mybir.dt.float8e4
BF16_NP = ml_dtypes.bfloat16
E4_NP = ml_dtypes.float8_e4m3
AF = mybir.ActivationFunctionType
DR = mybir.MatmulPerfMode.DoubleRow

S = 2048          # sequence length (per batch)
E = 1024          # embedding dim
HL = 8            # heads per core
D = 64            # head dim
TT = S // 128     # 16 token tiles
NG = 4            # DoubleRow contraction groups of 256 over E
NCH = S // 1024   # 2 q-chunks of 1024
WS = 32.0          # weight pre-scale: q/k/v (and A^T, w_proj) carried 32x
EXP_SCALE = 0.125 / (WS * WS)
PRIO_OFFSET = 800  # attention body scheduled ahead of filler work
SCORES_FP8 = True


def _install_drain_fix():
    """walrus in this container rejects the Tile kernel-tail Drain when it
    carries all semaphore waits on one instruction ("Too many sync wait
    commands"). Emit one wait_ge per semaphore, then a bare drain."""
    if getattr(tile.TileContext, "_drain_fix_installed", False):
        return

    def _split_drain_and_barrier(self, tick_clock, wait_clock):
        nc = self.nc
        probe = mybir.InstDrain(
            name="probe-drain", engine=mybir.EngineType.SP, ins=[], outs=[]
        )
        wait_clock.add_sem_waits(probe, ScopedClock({None: tick_clock.global_clock}))
        waits = re.findall(r"wait:S\[([A-Za-z0-9_]+)\]>=(\d+)", probe.concise())
        handles = {h.name: h for h in self.sems.allocated().values()}
        for name, val in waits:
            nc.sync.wait_ge(handles[name], int(val))
        nc.sync.drain()
        nc.all_engine_barrier()
        popped = nc._tile_sem_poison_stack.pop()
        assert popped is self._sem_poison
        nc.clear_and_free_semaphores(list(self.sems.allocated().values()))
        nc.all_engine_barrier()

    tile.TileContext._drain_and_barrier = _split_drain_and_barrier
    tile.TileContext._drain_fix_installed = True


def _emit(nc, tc, ctx):
    xh_d = nc.declare_dram_parameter("xh", [128, NG, 2, S], F8, isOutput=False)
    xl_d = nc.declare_dram_parameter("xl", [128, NG, 2, S], F8, isOutput=False)
    wqh_d = nc.declare_dram_parameter("wqh", [128, NG, 2, 1024], F8, isOutput=False)
    wql_d = nc.declare_dram_parameter("wql", [128, NG, 2, 1024], F8, isOutput=False)
    wvh_d = nc.declare_dram_parameter("wvh", [128, NG, 2, 512], F8, isOutput=False)
    wvl_d = nc.declare_dram_parameter("wvl", [128, NG, 2, 512], F8, isOutput=False)
    wph_d = nc.declare_dram_parameter("wph", [128, 2, 2, E], F8, isOutput=False)
    wpl_d = nc.declare_dram_parameter("wpl", [128, 2, 2, E], F8, isOutput=False)
    bqk_d = nc.declare_dram_parameter("bqk", [128, 8], F32, isOutput=False)
    bva_d = nc.declare_dram_parameter("bva", [1, 512], BF16, isOutput=False)
    out_d = nc.declare_dram_parameter("out", [S, E], BF16, isOutput=True)
    # tail-region (rows 1024:2048) c_proj partials, one per ct-PAIR;
    # summed on the host together with the core-pair reduction
    out2_d = nc.declare_dram_parameter("out2", [2, 1024, E], BF16, isOutput=True)

    consts = ctx.enter_context(tc.tile_pool(name="consts", bufs=1))
    statics = ctx.enter_context(tc.tile_pool(name="statics", bufs=1))
    ptp = ctx.enter_context(tc.tile_pool(name="ptp", bufs=14))
    rp = ctx.enter_context(tc.tile_pool(name="rp", bufs=6))
    yp = ctx.enter_context(tc.tile_pool(name="yp", bufs=4))
    # PSUM budget (8 banks):
    #   psS 2x[128,1024] = 4 (score tiles: depth 2 so scores(kt+1) overlaps
    #                         exp(kt) — the ACT exp stream must never starve)
    #   paP 2x[128,8,64] = 2 (reoriented attn@V accumulators, 1 bank each)
    #   dnP 1x[128,8]    = 1 (softmax denominators)
    #   psQ 1x[128,512]  = 1 (qkv / c_proj groups + A^T transpose staging)
    psS = ctx.enter_context(tc.tile_pool(name="psS", bufs=2, space="PSUM"))
    paP = ctx.enter_context(tc.tile_pool(name="paP", bufs=2, space="PSUM"))
    dnP = ctx.enter_context(tc.tile_pool(name="dnP", bufs=1, space="PSUM"))
    psQ = ctx.enter_context(tc.tile_pool(name="psQ", bufs=1, space="PSUM"))

    # ---- front section: DMA order matters (the DMA engines are a single
    # serialized resource). The hh-term operands first so qkv starts early,
    # then the lo tensors, wva, and wp (needed last) at the end ----
    xh_sb = statics.tile([128, NG, 2, S], F8)
    xl_sb = statics.tile([128, NG, 2, S], F8)
    wqh_sb = statics.tile([128, NG, 2, 1024], F8)
    wql_sb = statics.tile([128, NG, 2, 1024], F8)
    wvh_sb = statics.tile([128, NG, 2, 512], F8)
    wvl_sb = statics.tile([128, NG, 2, 512], F8)
    wph_sb = statics.tile([128, 2, 2, E], F8)
    wpl_sb = statics.tile([128, 2, 2, E], F8)

    for g in range(NG):
        nc.gpsimd.dma_start(out=wqh_sb[:, g, :, :], in_=wqh_d[:, g, :, :])
        if g == 0:
            nc.sync.dma_start(out=xh_sb[:, 0, 0, :], in_=xh_d[:, 0, 0, :])
            nc.sync.dma_start(out=xh_sb[:, 0, 1, :], in_=xh_d[:, 0, 1, :])
        else:
            nc.sync.dma_start(out=xh_sb[:, g, :, :], in_=xh_d[:, g, :, :])
    for g in range(NG):
        nc.gpsimd.dma_start(out=wql_sb[:, g, :, :], in_=wql_d[:, g, :, :])
    # xl rides both queues so the lh split-terms unlock ~3us earlier
    for g in range(NG):
        (nc.sync if g < 2 else nc.gpsimd).dma_start(
            out=xl_sb[:, g, :, :], in_=xl_d[:, g, :, :]
        )
    for g in range(NG):
        nc.gpsimd.dma_start(out=wvh_sb[:, g, :, :], in_=wvh_d[:, g, :, :])
        nc.gpsimd.dma_start(out=wvl_sb[:, g, :, :], in_=wvl_d[:, g, :, :])
    for g in range(2):
        nc.gpsimd.dma_start(out=wph_sb[:, g, :, :], in_=wph_d[:, g, :, :])
        nc.gpsimd.dma_start(out=wpl_sb[:, g, :, :], in_=wpl_d[:, g, :, :])

    bqk_sb = consts.tile([128, 8], F32)
    nc.sync.dma_start(out=bqk_sb[:, :], in_=bqk_d[:, :])
    bva_sb = consts.tile([1, 512], BF16)
    nc.sync.dma_start(out=bva_sb, in_=bva_d[:])
    ones_sb = consts.tile([1, 512], BF16)
    nc.gpsimd.memset(ones_sb[:], 1.0)
    # denominator column: value 1.0 makes pa*(1/den) carry 32*A (the V 32x
    # scale survives), which is exactly the e4m3 range A^T wants
    dcol_sb = consts.tile([128, 1], BF16)
    nc.gpsimd.memset(dcol_sb[:], 1.0)
    # identity for PE transposes
    ident_sb = consts.tile([128, 128], BF16)
    nc.gpsimd.memset(ident_sb[:], 1.0)
    nc.gpsimd.affine_select(
        out=ident_sb[:],
        in_=ident_sb[:],
        compare_op=mybir.AluOpType.is_equal,
        fill=0.0,
        base=0,
        pattern=[[1, 128]],
        channel_multiplier=-1,
    )

    # ---- qkv Q^T,K^T (W stationary, fp8 DR hi/lo x hi/lo, 32-scaled) paired
    # so head h's Q and K m-tiles arrive together, interleaved with V tiles ----
    qkt_sb = statics.tile([128, 8, S], F8)
    # one [64,...] tile per head pair: head h at partition 32*(h%2),
    # dim1 = Q/K, dim2 = the DoubleRow d-pair slot
    qk2_sb = [statics.tile([64, 2, 2, S], F8, name=f"qk2_{j}") for j in range(4)]
    va_sb = statics.tile([128, TT, 512], BF16)

    # split-product terms in DMA-arrival order: hh first, then hl, lh
    QK_TERMS = ((wqh_sb, xh_sb), (wql_sb, xh_sb), (wqh_sb, xl_sb))
    V_TERMS = ((wvh_sb, xh_sb), (wvl_sb, xh_sb), (wvh_sb, xl_sb))

    def emit_qk(m, ramp=False):
        # during the DMA-paced ramp the attention PSUM banks are still free:
        # spread the first pair's groups across them so more accumulations
        # are in flight per arriving weight tile
        if ramp == "attn":
            pools = [psQ, paP, paP, dnP]
        elif ramp:
            pools = [psS, psS, psS, psS]
        else:
            pools = [psQ] * 4
        tags = {id(psQ): "qk", id(paP): "o", id(dnP): "d", id(psS): "sc"}
        for tch in range(4):
            pqk = pools[tch].tile([128, 512], F32, tag=tags[id(pools[tch])])
            n = 0
            for wsb, xsb in QK_TERMS:
                for g in range(NG):
                    for c in range(2):
                        nc.tensor.matmul(
                            pqk[:, c * 256 : (c + 1) * 256],
                            wsb[:, g, :, m * 128 : (m + 1) * 128],
                            xsb[:, g, :, tch * 512 + c * 256 : tch * 512 + (c + 1) * 256],
                            start=(n == 0),
                            stop=(n == 23),
                            perf_mode=DR,
                        )
                        n += 1
            nc.vector.tensor_scalar_add(
                qkt_sb[:, m, tch * 512 : (tch + 1) * 512], pqk, bqk_sb[:, m : m + 1]
            )

    def emit_regroup(h):
        # partition-regrouping DMAs for the [32,2] d-packed score operands
        po, pr = (h % 2) * 64, 32 * (h % 2)
        t = qk2_sb[h // 2]
        for m, g in ((h // 2, 0), (4 + h // 2, 1)):
            for i in (0, 1):
                nc.sync.dma_start(
                    out=t[pr : pr + 32, g, i, :],
                    in_=qkt_sb[po + 32 * i : po + 32 * i + 32, m, :],
                )

    def emit_v(i, pool=None):
        # never the psS pool: its buf rotation would gate the first score
        # tiles behind the (wv-DMA-gated) V fills
        pool = pool or psQ
        tag = {id(psQ): "qk", id(paP): "o", id(dnP): "d", id(psS): "sc"}[id(pool)]
        pv1 = pool.tile([128, 512], F32, tag=tag)
        n = 0
        for wsb, xsb in V_TERMS:
            for g in range(NG):
                for c in range(2):
                    nc.tensor.matmul(
                        pv1[:, c * 256 : (c + 1) * 256],
                        xsb[:, g, :, i * 128 : (i + 1) * 128],
                        wsb[:, g, :, c * 256 : (c + 1) * 256],
                        start=(n == 0),
                        stop=False,
                        perf_mode=DR,
                    )
                    n += 1
        # V bias as a rank-1 ones-row matmul into the same PSUM group
        for c in range(2):
            nc.tensor.matmul(
                pv1[:, c * 256 : (c + 1) * 256],
                ones_sb[0:1, 0:128],
                bva_sb[0:1, c * 256 : (c + 1) * 256],
                start=False,
                stop=(c == 1),
            )
        nc.vector.tensor_copy(va_sb[:, i, :], pv1)

    # Q/K pairs 0-2 and V tiles 0-7 up front (the hh/hl split terms fill the
    # early-DMA window); pair 3 is deferred into the chunk-0 head loop as PE
    # filler. V0/V1 borrow the score banks (drained before the first score
    # tile's turn in the rotation); V2-7 stream through psQ and may lag —
    # attn@V catches up behind the exp stream.
    emit_qk(0, ramp="attn")    # pair 0 borrows the attention banks
    emit_qk(4, ramp="attn")
    if SCORES_FP8:
        emit_regroup(0)
        emit_regroup(1)
    emit_qk(1, ramp=True)      # pair 1 borrows the score banks
    emit_qk(5, ramp=True)
    if SCORES_FP8:
        emit_regroup(2)
        emit_regroup(3)
    for i in range(TT):
        emit_v(i)

    # ---- attention (q-chunks of 1024), interleaved with c_proj halves ----
    # A^T (32-scaled): bf16 master + e4m3 hi/lo pair for the DR c_proj
    at32_sb = statics.tile([128, 4, S], BF16)
    ath_sb = statics.tile([128, 4, S], F8)
    atl_sb = statics.tile([128, 4, S], F8)
    # [q, c]-oriented normalized attn out per head pair (double-buffered)
    asb_sb = statics.tile([128, 2, NCH, 8, 128], BF16)

    CP_TERMS = ((ath_sb, wph_sb), (ath_sb, wpl_sb), (atl_sb, wph_sb))

    def emit_cproj(i, last=False):
        ysb = yp.tile([128, E], BF16, tag="y")
        for ech in range(2):
            if last:
                # epilogue: attention banks are free — spread the groups
                pool = (paP, psS)[(2 * i + ech) % 2]
                py = pool.tile([128, 512], F32, tag="o" if pool is paP else "sc")
            else:
                py = psQ.tile([128, 512], F32, tag="qk")
            n = 0
            for asb, wsb in CP_TERMS:
                for g in range(2):
                    for c in range(2):
                        nc.tensor.matmul(
                            py[:, c * 256 : (c + 1) * 256],
                            asb[:, 2 * g : 2 * g + 2, i * 128 : (i + 1) * 128],
                            wsb[:, g, :, ech * 512 + c * 256 : ech * 512 + (c + 1) * 256],
                            start=(n == 0),
                            stop=(n == 11),
                            perf_mode=DR,
                        )
                        n += 1
            if last and ech == 0:
                nc.scalar.copy(out=ysb[:, ech * 512 : (ech + 1) * 512], in_=py)
            else:
                nc.vector.tensor_copy(ysb[:, ech * 512 : (ech + 1) * 512], py)
            nc.sync.dma_start(
                out=out_d[i * 128 : (i + 1) * 128, ech * 512 : (ech + 1) * 512],
                in_=ysb[:, ech * 512 : (ech + 1) * 512],
            )

    def emit_cproj_tail(ctp, last=False):
        # tail-region (q rows 1024:2048) partial for ct-pair ctp: contraction
        # is one DR group (256 rows), host sums the 2 partials. The final
        # tail is a pure epilogue: attention banks (paP) are free by then and
        # ACT is exp-idle, so spread psum groups and drains across both.
        for i in range(8, 16):
            y2 = yp.tile([128, E], BF16, tag="y")
            for ech in range(2):
                if last:
                    pool = (paP, psS)[(2 * i + ech) % 2]
                    py = pool.tile(
                        [128, 512], F32, tag="o" if pool is paP else "sc"
                    )
                else:
                    py = psQ.tile([128, 512], F32, tag="qk")
                n = 0
                for asb, wsb in CP_TERMS:
                    for c in range(2):
                        nc.tensor.matmul(
                            py[:, c * 256 : (c + 1) * 256],
                            asb[:, 2 * ctp : 2 * ctp + 2, i * 128 : (i + 1) * 128],
                            wsb[:, ctp, :, ech * 512 + c * 256 : ech * 512 + (c + 1) * 256],
                            start=(n == 0),
                            stop=(n == 5),
                            perf_mode=DR,
                        )
                        n += 1
                if last and (i + ech) % 2 == 0:
                    nc.scalar.copy(out=y2[:, ech * 512 : (ech + 1) * 512], in_=py)
                else:
                    nc.vector.tensor_copy(y2[:, ech * 512 : (ech + 1) * 512], py)
            nc.sync.dma_start(
                out=out2_d[ctp, (i - 8) * 128 : (i - 7) * 128, :], in_=y2
            )

    def emit_transposes(hp, j):
        # rebuild A^T for a completed (head pair, chunk): PE transpose per
        # q-tile, DVE drains the bf16 master, Pool (SBUF-only) derives the
        # e4m3 hi/lo pair c_proj consumes
        ctx_t = tc.high_priority(offset=PRIO_OFFSET)
        ctx_t.__enter__()
        for qc in range(8):
            qs = j * 1024 + qc * 128
            pst = (dnP if j == 0 else psQ).tile(
                [128, 128], BF16, tag="d" if j == 0 else "qk"
            )
            nc.tensor.transpose(pst, asb_sb[:, hp % 2, j, qc, :], ident_sb)
            nc.vector.tensor_copy(at32_sb[:, hp, qs : qs + 128], pst)
            nc.gpsimd.tensor_copy(
                ath_sb[:, hp, qs : qs + 128], at32_sb[:, hp, qs : qs + 128]
            )
            nc.gpsimd.tensor_sub(
                atl_sb[:, hp, qs : qs + 128],
                at32_sb[:, hp, qs : qs + 128],
                ath_sb[:, hp, qs : qs + 128],
            )
        ctx_t.__exit__(None, None, None)

    # head-major, chunks inner: chunk-1's long exp streams overlap the
    # qkv-heavy prefix so ACT (the bottleneck engine) never starves
    for h in range(HL):
        for j in range(NCH):
            q0 = j * 1024
            nkt = 8 * (j + 1)
            po = (h % 2) * 64
            qm, km = h // 2, 4 + h // 2
            ctx_hp = tc.high_priority(offset=PRIO_OFFSET)
            ctx_hp.__enter__()
            # one bank holds all 8 q-tile accumulators [128, qc, 64];
            # denominators accumulate per q-tile column in dnP
            pa = paP.tile([128, 8, 64], F32, tag="o")
            den = dnP.tile([128, 8], F32, tag="d")
            for kt in range(nkt):
                p = kt - 8 * j
                off = max(0, p * 128)
                qc0 = off // 128
                ps2 = psS.tile([128, 1024], F32, tag="sc")
                if SCORES_FP8:
                    t, pr = qk2_sb[h // 2], 32 * (h % 2)
                    aa = off
                    while aa < 1024:
                        bb = min(1024, (aa // 256 + 1) * 256)
                        nc.tensor.matmul(
                            ps2[:, aa:bb],
                            t[pr : pr + 32, 1, :, kt * 128 : (kt + 1) * 128],
                            t[pr : pr + 32, 0, :, q0 + aa : q0 + bb],
                            start=True,
                            stop=True,
                            perf_mode=DR,
                        )
                        aa = bb
                else:
                    for a, b in ([(off, 512), (512, 1024)] if off < 512 else [(off, 1024)]):
                        nc.tensor.matmul(
                            ps2[:, a:b],
                            qkt_sb[po : po + 64, km, kt * 128 : (kt + 1) * 128],
                            qkt_sb[po : po + 64, qm, q0 + a : q0 + b],
                            start=True,
                            stop=True,
                        )
                pt = ptp.tile([128, 1024], BF16, tag="pt")
                nc.scalar.activation(
                    out=pt[:, off:1024], in_=ps2[:, off:1024], func=AF.Exp,
                    scale=EXP_SCALE,
                )
                if p >= 0:
                    # causal triangle on the diagonal 128-block: keep where
                    # q >= k, zero elsewhere (Pool engine; ACT is exp-bound)
                    nc.gpsimd.affine_select(
                        out=pt[:, off : off + 128],
                        in_=pt[:, off : off + 128],
                        compare_op=mybir.AluOpType.is_ge,
                        fill=0.0,
                        base=0,
                        pattern=[[1, 128]],
                        channel_multiplier=-1,
                    )
                # reoriented attn@V: out[q, d] — P stationary, V moving
                # (64 cols), one matmul per live q-tile; denominator via a
                # rank-1 ones column into the shared den bank
                for qc in range(qc0, 8):
                    qs = qc * 128
                    nc.tensor.matmul(
                        pa[:, qc, :],
                        pt[:, qs : qs + 128],
                        va_sb[:, kt, h * 64 : (h + 1) * 64],
                        start=(kt == 0 and qc == 0),
                        stop=(kt == 8 * j + qc),
                        skip_group_check=True,
                    )
                    nc.tensor.matmul(
                        den[:, qc : qc + 1],
                        pt[:, qs : qs + 128],
                        dcol_sb[:, :],
                        start=(kt == 0 and qc == 0),
                        stop=(kt == 8 * j + qc),
                        skip_group_check=True,
                    )
            rinv = rp.tile([128, 8], F32, tag="ri")
            nc.vector.reciprocal(out=rinv, in_=den)
            # normalization folded into the PSUM drain: asb[q, c] = pa * rinv
            # (per-partition scalar). GPSIMD can't touch PSUM, so DVE only.
            for qc in range(8):
                nc.vector.tensor_scalar_mul(
                    asb_sb[:, (h // 2) % 2, j, qc, po : po + 64],
                    pa[:, qc, :],
                    rinv[:, qc : qc + 1],
                )
            ctx_hp.__exit__(None, None, None)
            if h == 7 and j == 0:
                # full chunk-0 A^T is complete: c_proj rows 0:1024 can run
                # as PE filler under head 7's chunk-1 exp stream. The hp3
                # transposes must come first.
                emit_transposes(3, 0)
                for i in range(4):
                    emit_cproj(i)
        if h % 2 == 1:
            hp = h // 2
            if not (h == 7):
                emit_transposes(hp, 0)
            emit_transposes(hp, 1)
        # deferred qkv as PE filler (pair 3 for heads 6-7); the c_proj
        # tail halves run as soon as their A^T column blocks complete
        if h == 0:
            emit_qk(2)
            emit_qk(6)
            if SCORES_FP8:
                emit_regroup(4)
                emit_regroup(5)
        elif h == 1:
            emit_qk(3)
            emit_qk(7)
            if SCORES_FP8:
                emit_regroup(6)
                emit_regroup(7)
        elif h == 3:
            emit_cproj_tail(0)
    for i in range(4, 8):
        emit_cproj(i, last=True)
    emit_cproj_tail(1, last=True)


def build_nc():
    _install_drain_fix()
    from contextlib import ExitStack

    nc = bacc.Bacc()
    with ExitStack() as ctx:
        tc = ctx.enter_context(tile.TileContext(nc))
        _emit(nc, tc, ctx)
    nc.finalize()  # Bacc: alloc_regs + insert_library_loads happen here
    return nc


def _split_pack(a, scale, ng):
    """Split f32 array [rows, cols] into e4m3 (hi, lo) of scale*a, packed
    [128, ng, 2, cols] so DR group g, slot i, partition p holds row
    256*g + 128*i + p."""
    a = np.asarray(a, dtype=np.float32) * scale
    hi = a.astype(E4_NP)
    lo = (a - hi.astype(np.float32)).astype(E4_NP)
    cols = a.shape[1]

    def pack(x):
        return np.ascontiguousarray(
            x.reshape(ng, 2, 128, cols).transpose(2, 0, 1, 3)
        )

    return pack(hi), pack(lo)


def make_in_maps(inputs, w_attn, b_attn, w_proj, b_proj):
    """Build the 8 per-core input dicts from the full tensors."""
    x = np.asarray(inputs, dtype=np.float32)
    w_attn = np.asarray(w_attn, dtype=np.float32)
    b_attn = np.asarray(b_attn, dtype=np.float32)
    w_proj = np.asarray(w_proj, dtype=np.float32)

    # X^T splits are per batch (shared by the core pair)
    xsp = [_split_pack(x[b].T, 1.0, NG) for b in range(4)]

    in_maps = []
    for c in range(8):
        b, half = c // 2, c % 2
        h0 = half * 8
        cols = np.arange(h0 * 64, h0 * 64 + 512)
        wqk = np.concatenate([w_attn[:, cols], w_attn[:, 1024 + cols]], axis=1)
        wqh, wql = _split_pack(wqk, WS, NG)
        bqk = np.ascontiguousarray(
            (WS * np.concatenate([b_attn[cols], b_attn[1024 + cols]]))
            .reshape(8, 128)
            .T
        )
        vbase = 2048 + h0 * 64
        wvh, wvl = _split_pack(w_attn[:, vbase : vbase + 512], WS, NG)
        bva = (WS * b_attn[vbase : vbase + 512]).reshape(1, 512)
        wph, wpl = _split_pack(w_proj[h0 * 64 : h0 * 64 + 512, :], WS, 2)
        in_maps.append(
            {
                "xh": xsp[b][0],
                "xl": xsp[b][1],
                "wqh": wqh,
                "wql": wql,
                "wvh": wvh,
                "wvl": wvl,
                "wph": wph,
                "wpl": wpl,
                "bqk": np.ascontiguousarray(bqk.astype(np.float32)),
                "bva": np.ascontiguousarray(bva.astype(BF16_NP)),
            }
        )
    return in_maps


_CACHE = {}


def kernel(**inputs):
    nc = _CACHE.get("nc")
    if nc is None:
        nc = _CACHE["nc"] = build_nc()
    in_maps = make_in_maps(
        inputs["inputs"],
        inputs["w_attn"],
        inputs["b_attn"],
        inputs["w_proj"],
        inputs["b_proj"],
    )
    res = run_bass_kernel_spmd(nc, in_maps, core_ids=list(range(8)))
    return gather(res.results, inputs["b_proj"])


def gather(results, b_proj):
    # device output carries the (32*A)·(32*Wp) = 1024x weight scale
    out = np.zeros((4, S, E), dtype=np.float32)
    for b in range(4):
        for c in (2 * b, 2 * b + 1):
            r = results[c]
            # rows 0:1024 come from "out"; the device writes rows 1024:2048
            # only via the per-ct-pair partials in "out2"
            out[b, 0:1024] += r["out"][0:1024].astype(np.float32)
            out[b, 1024:2048] += r["out2"].astype(np.float32).sum(axis=0)
    out *= 1.0 / (WS * WS)
    out += np.asarray(b_proj, dtype=np.float32)[None, None, :]
    return out
